# revision 16
# baseline (speedup 1.0000x reference)
"""Trainium2 Bass kernel for nn_AstroSymbolicEpisodicLayer.

Strategy
--------
8 cores = (batch b in 0..3) x (query-half h in 0..1). One SPMD program; all
per-core differences are data (each core's qT has its own query half first).

Math decomposition (validated to 1.4e-6 in fp32 vs the jax reference):
 - Circular-convolution binding via a packed real DFT implemented as two
   1024x1024 matmuls (F forward, G inverse).  Packed layout: rows 0..511 are
   Re bins 0..511, row 512 is Re bin 512, rows 513..1023 are Im bins 1..511.
   Pointwise complex multiply pairs row f with row 512+f (tile j with tile
   j+4), plus a 2-row fixup for the purely-real bins 0 and 512.
 - rfft(role_vecs) = role_weights @ rfft(role_norm) by linearity; the softmax
   denominator cancels in the K_bound l2-normalization so we use plain exp.
 - Per-token normalizations are folded: 1/||K_bound|| becomes a per-partition
   scalar in the scores epilogue (scores computed keys-major), 1/||Q|| scales
   q_n in SBUF, and the attention row-sum is folded into the output stage.

Everything is feature-major ("xT" = [feature, token]) so every chained matmul
is out = lhsT.T @ rhs with contraction on the partition dim.
"""

import sys

for _p in ("/opt/trn_rl_repo", "/root/.axon_site/_ro/trn_rl_repo"):
    if _p not in sys.path:
        sys.path.append(_p)

import numpy as np
import ml_dtypes

import concourse.bass as bass
import concourse.bacc as bacc
import concourse.tile as tile
from concourse import mybir
from concourse.bass_utils import run_bass_kernel_spmd

BF16 = ml_dtypes.bfloat16
F32 = mybir.dt.float32
BF = mybir.dt.bfloat16

B, TQ, TK, D, R, H = 4, 2048, 2048, 1024, 64, 512
TAU_BASE, ASTRO_DECAY = 1.0, 0.95
P = 128
NJ = D // P            # 8 feature chunks
NT = TK // 512         # 4 token chunks of 512
TQH = TQ // 2          # per-core query rows (1024)
NQC = TQH // 512       # 2 query chunks
NKT = TK // P          # 16 key tiles of 128
AF = mybir.ActivationFunctionType
ALU = mybir.AluOpType


# ---------------------------------------------------------------- host consts
def _build_dft_mats(n=1024):
    j = np.arange(n)[:, None].astype(np.float64)
    f = np.arange(n // 2)[None, :].astype(np.float64)
    F = np.zeros((n, n), np.float64)
    F[:, :512] = np.cos(2 * np.pi * j * f / n)
    F[:, 512] = np.cos(np.pi * j[:, 0])
    fi = np.arange(1, 512)[None, :].astype(np.float64)
    F[:, 513:] = -np.sin(2 * np.pi * j * fi / n)

    G = np.zeros((n, n), np.float64)
    d = np.arange(n)[None, :].astype(np.float64)
    G[0, :] = 1.0 / n
    ff = np.arange(1, 512)[:, None].astype(np.float64)
    G[1:512, :] = 2.0 * np.cos(2 * np.pi * ff * d / n) / n
    G[512, :] = ((-1.0) ** d[0]) / n
    G[513:, :] = -2.0 * np.sin(2 * np.pi * ff * d / n) / n
    return F.astype(np.float32), G.astype(np.float32)


def _pack_rfft(x):
    X = np.fft.rfft(np.asarray(x, np.float64), axis=-1)
    out = np.empty(x.shape, np.float32)
    out[..., :512] = X.real[..., :512]
    out[..., 512] = X.real[..., 512]
    out[..., 513:] = X.imag[..., 1:512]
    return out


# ---------------------------------------------------------------- bass kernel
def _emit(nc: bass.Bass):
    # ---- I/O declarations (names = in_map keys) ----
    din = {}
    for nm, shp, dt in [
        ("qT", [D, TQ], BF), ("kT", [D, TK], BF), ("vT", [D, TK], BF),
        ("wq", [D, D], BF), ("wk", [D, D], BF), ("wv", [D, D], BF),
        ("wo", [D, D], BF), ("fmat", [D, D], BF), ("gmat", [D, D], BF),
        ("wr1", [D, H], BF), ("wr2", [H, R], BF), ("pf", [R, D], BF),
        ("bq", [P, NJ], F32), ("bk", [P, NJ], F32), ("bo", [P, NJ], F32),
        ("bv_row", [1, D], F32), ("br1", [P, H // P], F32),
        ("scal", [1, 2], F32),
    ]:
        din[nm] = nc.declare_dram_parameter(nm, shp, dt, isOutput=False)
    outT = nc.declare_dram_parameter("outT", [D, TQH], F32, isOutput=True)
    ns_out = nc.declare_dram_parameter("ns_out", [1, 1], F32, isOutput=True)

    # internal DRAM spill tensors
    KT_d = nc.dram_tensor("KT_d", [D, TK], BF)
    Kf_d = nc.dram_tensor("Kf_d", [D, TK], BF)
    Rf_d = nc.dram_tensor("Rf_d", [D, TK], BF)
    kn_d = nc.dram_tensor("kn_d", [D, TK], BF)
    V_d = nc.dram_tensor("V_d", [TK, D], BF)

    c3 = lambda t: t.rearrange("(po pi) x -> pi po x", pi=P)

    with tile.TileContext(nc) as tc, \
         tc.tile_pool(name="w", bufs=2) as wpool, \
         tc.tile_pool(name="wsmall", bufs=1) as wsmall, \
         tc.tile_pool(name="instream", bufs=2) as instream, \
         tc.tile_pool(name="outstage", bufs=4) as outstage, \
         tc.tile_pool(name="persist", bufs=1) as persist, \
         tc.tile_pool(name="cmul", bufs=4) as cmulpool, \
         tc.tile_pool(name="zstage", bufs=2) as zstage, \
         tc.tile_pool(name="scores", bufs=1) as scorepool, \
         tc.tile_pool(name="attst", bufs=2) as attst, \
         tc.tile_pool(name="ctxp", bufs=2) as ctxpool, \
         tc.tile_pool(name="tmp", bufs=2) as tmp, \
         tc.tile_pool(name="cmtmp", bufs=2) as cmtmp, \
         tc.tile_pool(name="small", bufs=1) as small, \
         tc.tile_pool(name="rows", bufs=2) as rows, \
         tc.tile_pool(name="ps", bufs=4, space="PSUM") as ps, \
         tc.tile_pool(name="psrow", bufs=3, space="PSUM") as psrow:

        # ---- constants ----
        ones_k = small.tile([P, 1], BF, tag="ones_k")      # lhsT for partition sums
        nc.vector.memset(ones_k, 1.0)
        ones_b = small.tile([1, P], F32, tag="ones_b")     # lhsT for row broadcast
        nc.vector.memset(ones_b, 1.0)
        bq_sb = small.tile([P, NJ], F32, tag="bq")
        nc.sync.dma_start(bq_sb, din["bq"][:])
        bk_sb = small.tile([P, NJ], F32, tag="bk")
        nc.sync.dma_start(bk_sb, din["bk"][:])
        bo_sb = small.tile([P, NJ], F32, tag="bo")
        nc.sync.dma_start(bo_sb, din["bo"][:])
        br1_sb = small.tile([P, H // P], F32, tag="br1")
        nc.sync.dma_start(br1_sb, din["br1"][:])
        scal_sb = small.tile([1, 2], F32, tag="scal")
        nc.sync.dma_start(scal_sb, din["scal"][:])
        bv_bc = small.tile([P, D], F32, tag="bv_bc")       # bv broadcast to 128 rows
        nc.sync.dma_start(
            bv_bc,
            bass.AP(tensor=din["bv_row"], offset=0, ap=[[0, P], [1, D]]),
        )

        # pre-touch const tiles so later consumers don't accumulate one sync
        # wait per DMA queue on top of their compute waits (ACT has a small
        # hardware wait table)
        pre_s = small.tile([P, 1], F32, tag="pre_s")
        nc.scalar.activation(pre_s, bq_sb[:, 0:1], AF.Identity, bias=bk_sb[:, 0:1])
        nc.scalar.activation(pre_s, pre_s, AF.Relu, bias=br1_sb[:, 0:1])
        pre_v = small.tile([P, 1], F32, tag="pre_v")
        nc.vector.tensor_scalar_add(pre_v, bv_bc[:, 0:1], bo_sb[:, 0:1])
        pre_v2 = small.tile([1, 1], F32, tag="pre_v2")
        nc.vector.tensor_scalar_add(pre_v2, scal_sb[0:1, 0:1], 0.0)

        def load_w(name, shape3):
            t = wpool.tile(shape3, BF, tag="w")
            nc.sync.dma_start(t, c3(din[name][:]))
            return t

        # ============================================== S1: K = Wk.T @ k + bk
        wk = load_w("wk", [P, NJ, D])
        for t in range(NT):
            kin = instream.tile([P, NJ, 512], BF, tag="instream")
            nc.sync.dma_start(kin, c3(din["kT"][:])[:, :, t * 512:(t + 1) * 512])
            for j in range(NJ):
                pst = ps.tile([P, 512], F32, tag="mm")
                for i in range(NJ):
                    nc.tensor.matmul(pst, wk[:, i, j * P:(j + 1) * P], kin[:, i, :],
                                     start=(i == 0), stop=(i == NJ - 1))
                kt_o = outstage.tile([P, 512], BF, tag="outstage")
                nc.scalar.activation(kt_o, pst, AF.Identity, bias=bk_sb[:, j:j + 1])
                nc.sync.dma_start(KT_d[j * P:(j + 1) * P, t * 512:(t + 1) * 512], kt_o)

        # ============================================== S2: Kf = F.T @ K
        fmat = load_w("fmat", [P, NJ, D])
        for t in range(NT):
            ktc = instream.tile([P, NJ, 512], BF, tag="instream")
            nc.sync.dma_start(ktc, c3(KT_d[:])[:, :, t * 512:(t + 1) * 512])
            for j in range(NJ):
                pst = ps.tile([P, 512], F32, tag="mm")
                for i in range(NJ):
                    nc.tensor.matmul(pst, fmat[:, i, j * P:(j + 1) * P], ktc[:, i, :],
                                     start=(i == 0), stop=(i == NJ - 1))
                kf_o = outstage.tile([P, 512], BF, tag="outstage")
                nc.scalar.activation(kf_o, pst, AF.Copy)
                nc.sync.dma_start(Kf_d[j * P:(j + 1) * P, t * 512:(t + 1) * 512], kf_o)

        # ============================================== S3: role MLP -> Rf
        wr1 = wsmall.tile([P, NJ, H], BF, tag="wr1")
        nc.sync.dma_start(wr1, c3(din["wr1"][:]))
        hidden = scorepool.tile([P, H // P, TK], BF, tag="scT", name="hidden")
        for t in range(NT):
            ktc = instream.tile([P, NJ, 512], BF, tag="instream")
            nc.sync.dma_start(ktc, c3(KT_d[:])[:, :, t * 512:(t + 1) * 512])
            for jh in range(H // P):
                pst = ps.tile([P, 512], F32, tag="mm")
                for i in range(NJ):
                    nc.tensor.matmul(pst, wr1[:, i, jh * P:(jh + 1) * P], ktc[:, i, :],
                                     start=(i == 0), stop=(i == NJ - 1))
                nc.scalar.activation(hidden[:, jh, t * 512:(t + 1) * 512], pst,
                                     AF.Relu, bias=br1_sb[:, jh:jh + 1])
        wr2 = wsmall.tile([P, H // P, R], BF, tag="wr2")
        nc.sync.dma_start(wr2, c3(din["wr2"][:]))
        rw = persist.tile([R, TK], BF, tag="rw")
        for t in range(NT):
            ps64 = ps.tile([R, 512], F32, tag="mm")
            for i2 in range(H // P):
                nc.tensor.matmul(ps64, wr2[:, i2, :], hidden[:, i2, t * 512:(t + 1) * 512],
                                 start=(i2 == 0), stop=(i2 == H // P - 1))
            nc.scalar.activation(rw[:, t * 512:(t + 1) * 512], ps64, AF.Exp)
        pf = wsmall.tile([R, D], BF, tag="pf")
        nc.sync.dma_start(pf, din["pf"][:])
        for t in range(NT):
            for j in range(NJ):
                pst = ps.tile([P, 512], F32, tag="mm")
                nc.tensor.matmul(pst, pf[:, j * P:(j + 1) * P],
                                 rw[:, t * 512:(t + 1) * 512], start=True, stop=True)
                rf_o = outstage.tile([P, 512], BF, tag="outstage")
                nc.scalar.activation(rf_o, pst, AF.Copy)
                nc.sync.dma_start(Rf_d[j * P:(j + 1) * P, t * 512:(t + 1) * 512], rf_o)

        # ============================================== S4: Z = cmul(Kf, Rf)
        # packed pairing: row f (tile j) with row 512+f (tile j+4)
        Z_d = nc.dram_tensor("Z_d", [D, TK], BF)
        for j in range(4):
            alo = cmulpool.tile([P, TK], BF, tag="cmul")
            nc.sync.dma_start(alo, Kf_d[j * P:(j + 1) * P, :])
            ahi = cmulpool.tile([P, TK], BF, tag="cmul")
            nc.sync.dma_start(ahi, Kf_d[(j + 4) * P:(j + 5) * P, :])
            blo = cmulpool.tile([P, TK], BF, tag="cmul")
            nc.sync.dma_start(blo, Rf_d[j * P:(j + 1) * P, :])
            bhi = cmulpool.tile([P, TK], BF, tag="cmul")
            nc.sync.dma_start(bhi, Rf_d[(j + 4) * P:(j + 5) * P, :])
            t2 = cmtmp.tile([P, TK], BF, tag="cm_t2")
            zlo = zstage.tile([P, TK], BF, tag="zst")
            zhi = zstage.tile([P, TK], BF, tag="zst")
            nc.vector.tensor_mul(zlo, alo, blo)
            nc.vector.tensor_mul(t2, ahi, bhi)
            nc.vector.tensor_tensor(zlo, zlo, t2, ALU.subtract)
            nc.vector.tensor_mul(zhi, alo, bhi)
            nc.vector.tensor_mul(t2, ahi, blo)
            nc.vector.tensor_tensor(zhi, zhi, t2, ALU.add)
            if j == 0:
                # rows 0 (re_0) and 512 (re_512) are purely real
                nc.vector.tensor_mul(zlo[0:1, :], alo[0:1, :], blo[0:1, :])
                nc.vector.tensor_mul(zhi[0:1, :], ahi[0:1, :], bhi[0:1, :])
            nc.sync.dma_start(Z_d[j * P:(j + 1) * P, :], zlo)
            nc.sync.dma_start(Z_d[(j + 4) * P:(j + 5) * P, :], zhi)

        # ============================================== S5: KB = G.T @ Z, norms
        gmat = load_w("gmat", [P, NJ, D])
        rk_stage = nc.dram_tensor("rk_stage", [1, TK], F32)
        for t in range(NT):
            ztc = instream.tile([P, NJ, 512], BF, tag="instream")
            nc.sync.dma_start(ztc, c3(Z_d[:])[:, :, t * 512:(t + 1) * 512])
            psn = psrow.tile([1, 512], F32, tag="rowsum")
            for j in range(NJ):
                pst = ps.tile([P, 512], F32, tag="mm")
                for i in range(NJ):
                    nc.tensor.matmul(pst, gmat[:, i, j * P:(j + 1) * P],
                                     ztc[:, i, :],
                                     start=(i == 0), stop=(i == NJ - 1))
                kb_o = outstage.tile([P, 512], BF, tag="outstage")
                nc.scalar.activation(kb_o, pst, AF.Copy)
                nc.sync.dma_start(kn_d[j * P:(j + 1) * P, t * 512:(t + 1) * 512], kb_o)
                sq = tmp.tile([P, 512], BF, tag="sq")
                nc.vector.tensor_mul(sq, kb_o, kb_o)
                nc.tensor.matmul(psn, ones_k, sq, start=(j == 0), stop=(j == NJ - 1))
            # rk chunk = 1/sqrt(norm2) -> stage to DRAM for the column reload
            srow = rows.tile([1, 512], F32, tag="srow")
            nc.scalar.activation(srow, psn, AF.Sqrt)
            rrow = rows.tile([1, 512], F32, tag="rrow")
            nc.vector.reciprocal(rrow, srow)
            nc.sync.dma_start(rk_stage[0:1, t * 512:(t + 1) * 512], rrow)
        # rk as [128,16] (per-key-tile partition scalars), negated for the
        # (S*(-rk) + 1) epilogue
        rk_col = small.tile([P, NKT], F32, tag="rk_col")
        nc.sync.dma_start(
            rk_col, rk_stage.rearrange("o (kt p) -> (o p) kt", p=P))
        nrk_col = small.tile([P, NKT], F32, tag="nrk_col")
        nc.vector.tensor_scalar_mul(nrk_col, rk_col, -1.0)

        # ============================================== S6: V = vT.T-major proj
        wv = load_w("wv", [P, NJ, D])
        for to in range(NKT):
            vin = attst.tile([P, NJ, P], BF, tag="knstream", name="vin")
            nc.sync.dma_start(vin, c3(din["vT"][:])[:, :, to * P:(to + 1) * P])
            for n in range(2):
                pst = ps.tile([P, 512], F32, tag="mm")
                for i in range(NJ):
                    nc.tensor.matmul(pst, vin[:, i, :], wv[:, i, n * 512:(n + 1) * 512],
                                     start=(i == 0), stop=(i == NJ - 1))
                v_o = outstage.tile([P, 512], BF, tag="outstage")
                nc.vector.tensor_add(v_o, pst, bv_bc[:, n * 512:(n + 1) * 512])
                nc.sync.dma_start(V_d[to * P:(to + 1) * P, n * 512:(n + 1) * 512], v_o)

        # ============================================== S7: Q proj, tau, q_n
        wq = load_w("wq", [P, NJ, D])
        qn = persist.tile([P, NJ, TQH], BF, tag="qn")
        rq_bc = small.tile([P, TQH], F32, tag="rq_bc")
        ssc = small.tile([1, NT], F32, tag="ssc")
        for t in range(NT):  # over full TQ; chunks 0,1 are this core's half
            qin = instream.tile([P, NJ, 512], BF, tag="instream")
            nc.sync.dma_start(qin, c3(din["qT"][:])[:, :, t * 512:(t + 1) * 512])
            psn = psrow.tile([1, 512], F32, tag="rowsum")
            for j in range(NJ):
                pst = ps.tile([P, 512], F32, tag="mm")
                for i in range(NJ):
                    nc.tensor.matmul(pst, wq[:, i, j * P:(j + 1) * P], qin[:, i, :],
                                     start=(i == 0), stop=(i == NJ - 1))
                if t < NQC:
                    qv = qn[:, j, t * 512:(t + 1) * 512]
                else:
                    qv = tmp.tile([P, 512], BF, tag="qv_other")
                nc.scalar.activation(qv, pst, AF.Identity, bias=bq_sb[:, j:j + 1])
                sq = tmp.tile([P, 512], BF, tag="sq")
                nc.vector.tensor_mul(sq, qv, qv)
                nc.tensor.matmul(psn, ones_k, sq, start=(j == 0), stop=(j == NJ - 1))
            # sqrt of norms; accumulate sum(sqrt) for the surprise mean
            srow = rows.tile([1, 512], F32, tag="srow")
            nc.scalar.activation(srow, psn, AF.Sqrt, accum_out=ssc[0:1, t:t + 1])
            if t < NQC:  # own half: rq broadcast for q_n scaling
                rrow = rows.tile([1, 512], F32, tag="rrow")
                nc.vector.reciprocal(rrow, srow)
                rb_ps = ps.tile([P, 512], F32, tag="mm")
                nc.tensor.matmul(rb_ps, ones_b, rrow, start=True, stop=True)
                nc.vector.tensor_copy(rq_bc[:, t * 512:(t + 1) * 512], rb_ps)
        ss = small.tile([1, 1], F32, tag="ss")
        nc.vector.reduce_sum(ss, ssc, axis=mybir.AxisListType.X)
        # new_state = 0.95*astro + 0.05*ss/(32*2048)
        ns_t = small.tile([1, 1], F32, tag="ns_t")
        v1 = small.tile([1, 1], F32, tag="v1")
        nc.vector.tensor_scalar_mul(v1, ss, (1.0 - ASTRO_DECAY) / (32.0 * TQ))
        v2 = small.tile([1, 1], F32, tag="v2")
        nc.vector.tensor_scalar_mul(v2, scal_sb[0:1, 0:1], ASTRO_DECAY)
        nc.vector.tensor_add(ns_t, v1, v2)
        nc.sync.dma_start(ns_out[:], ns_t)
        # c = tau/4 = max(1 + astro_scale*ns, 0.001)/4
        c_t = small.tile([1, 1], F32, tag="c_t")
        nc.vector.tensor_mul(c_t, ns_t, scal_sb[0:1, 1:2])
        nc.vector.tensor_scalar(c_t, c_t, 1.0, 0.001, ALU.add, ALU.max)
        nc.vector.tensor_scalar_mul(c_t, c_t, 0.25 * TAU_BASE)
        cb_ps = ps.tile([P, 1], F32, tag="mm")
        nc.tensor.matmul(cb_ps, ones_b, c_t, start=True, stop=True)
        c_sb = small.tile([P, 1], F32, tag="c_sb")
        nc.vector.tensor_copy(c_sb, cb_ps)
        for j in range(NJ):
            nc.vector.tensor_mul(qn[:, j, :], qn[:, j, :], rq_bc)

        # ============================================== S8/S9: attention per qc
        wo = load_w("wo", [P, NJ, D])
        for qc in range(NQC):
            scT = scorepool.tile([P, NKT, 512], BF, tag="scT", name=f"scT{qc}")
            rs_ps = psrow.tile([1, 512], F32, tag="rowsum", name=f"rs_ps{qc}")
            for kt in range(NKT):
                knc = attst.tile([P, NJ, P], BF, tag="knstream")
                nc.sync.dma_start(knc, c3(kn_d[:])[:, :, kt * P:(kt + 1) * P])
                ps_s = ps.tile([P, 512], F32, tag="mm")
                for j in range(NJ):
                    nc.tensor.matmul(ps_s, knc[:, j, :], qn[:, j, qc * 512:(qc + 1) * 512],
                                     start=(j == 0), stop=(j == NJ - 1))
                u0 = tmp.tile([P, 512], BF, tag="u0")
                # u0 = 1 - cos = S*(-rk[key]) + 1
                nc.vector.tensor_scalar(u0, ps_s, nrk_col[:, kt:kt + 1], 1.0,
                                        ALU.mult, ALU.add)
                u1 = tmp.tile([P, 512], BF, tag="u1")
                nc.scalar.activation(u1, u0, AF.Square)
                u2 = tmp.tile([P, 512], BF, tag="u2")
                nc.vector.tensor_scalar_mul(u2, u1, c_sb)
                # scores = relu(1 - c*(1-cos)^2)
                nc.scalar.activation(scT[:, kt, :], u2, AF.Relu,
                                     bias=1.0, scale=-1.0)
                nc.tensor.matmul(rs_ps, ones_k, scT[:, kt, :],
                                 start=(kt == 0), stop=(kt == NKT - 1))
            # rowsum reciprocal -> broadcast [128,512]
            rs_row = small.tile([1, 512], F32, tag="rs_row", name=f"rs_row{qc}")
            nc.vector.reciprocal(rs_row, rs_ps)
            rb_ps = ps.tile([P, 512], F32, tag="mm")
            nc.tensor.matmul(rb_ps, ones_b, rs_row, start=True, stop=True)
            rs_bc = small.tile([P, 512], F32, tag="rs_bc", name=f"rs_bc{qc}")
            nc.vector.tensor_copy(rs_bc, rb_ps)

            # context
            ctx = ctxpool.tile([P, NJ, 512], BF, tag="ctx")
            for j2g in range(2):  # Dout groups of 512 (4 psum tiles each)
                pcs = [ps.tile([P, 512], F32, tag="mm", name=f"pcs{i}") for i in range(4)]
                for kt in range(NKT):
                    vkt = attst.tile([P, 512], BF, tag="vstream8")
                    nc.sync.dma_start(
                        vkt, V_d[kt * P:(kt + 1) * P, j2g * 512:(j2g + 1) * 512])
                    for jj in range(4):
                        nc.tensor.matmul(pcs[jj], vkt[:, jj * P:(jj + 1) * P],
                                         scT[:, kt, :],
                                         start=(kt == 0), stop=(kt == NKT - 1))
                for jj in range(4):
                    nc.scalar.activation(ctx[:, j2g * 4 + jj, :], pcs[jj], AF.Copy)
            # output projection for this q chunk
            for j in range(NJ):
                pst = ps.tile([P, 512], F32, tag="mm")
                for i in range(NJ):
                    nc.tensor.matmul(pst, wo[:, i, j * P:(j + 1) * P], ctx[:, i, :],
                                     start=(i == 0), stop=(i == NJ - 1))
                o_t = tmp.tile([P, 512], F32, tag="o_t")
                nc.vector.tensor_mul(o_t, pst, rs_bc)
                nc.vector.tensor_scalar_add(o_t, o_t, bo_sb[:, j:j + 1])
                nc.sync.dma_start(
                    outT[j * P:(j + 1) * P, qc * 512:(qc + 1) * 512], o_t)

    return nc


_CACHE = {}


def _get_nc():
    if "nc" not in _CACHE:
        nc = bacc.Bacc(None, target_bir_lowering=False)
        _emit(nc)
        nc.finalize()
        _CACHE["nc"] = nc
    return _CACHE["nc"]


def kernel(q_in, k_in, v_in, astrocyte_state, Wq, bq, Wk, bk, Wv, bv, Wo, bo,
           role_matrix, Wr1, br1, Wr2, astro_scale, **_ignored):
    nc = _get_nc()

    F, G = _build_dft_mats(D)
    role = np.asarray(role_matrix, np.float32)
    role = role / np.clip(np.linalg.norm(role, axis=-1, keepdims=True), 1e-12, None)
    PF = _pack_rfft(role)

    bf = lambda x: np.ascontiguousarray(np.asarray(x, np.float32)).astype(BF16)
    btile = lambda x: np.ascontiguousarray(
        np.asarray(x, np.float32).reshape(NJ, P).T)

    weights = {
        "wq": bf(Wq), "wk": bf(Wk), "wv": bf(Wv), "wo": bf(Wo),
        "fmat": bf(F), "gmat": bf(G), "wr1": bf(Wr1), "wr2": bf(Wr2),
        "pf": bf(PF),
        "bq": btile(bq), "bk": btile(bk), "bo": btile(bo),
        "bv_row": np.asarray(bv, np.float32).reshape(1, D),
        "br1": np.ascontiguousarray(np.asarray(br1, np.float32).reshape(H // P, P).T),
    }

    in_maps = []
    for core in range(8):
        b, h = core // 2, core % 2
        q = np.asarray(q_in[b], np.float32)
        own = q[h * TQH:(h + 1) * TQH]
        other = q[(1 - h) * TQH:(2 - h) * TQH]
        qTh = np.concatenate([own, other], axis=0).T  # own half first
        m = dict(weights)
        m["qT"] = np.ascontiguousarray(qTh).astype(BF16)
        m["kT"] = np.ascontiguousarray(np.asarray(k_in[b], np.float32).T).astype(BF16)
        m["vT"] = np.ascontiguousarray(np.asarray(v_in[b], np.float32).T).astype(BF16)
        m["scal"] = np.array(
            [[np.float32(astrocyte_state[b]),
              np.float32(np.asarray(astro_scale).reshape(-1)[0])]], np.float32)
        in_maps.append(m)

    res = run_bass_kernel_spmd(nc, in_maps, core_ids=list(range(8)))

    output = np.empty((B, TQ, D), np.float32)
    new_state = np.empty((B,), np.float32)
    for core in range(8):
        b, h = core // 2, core % 2
        output[b, h * TQH:(h + 1) * TQH, :] = res.results[core]["outT"].T
        if h == 0:
            new_state[b] = res.results[core]["ns_out"][0, 0]
    return output, new_state


# revision 17
# speedup vs baseline: 1.3259x; 1.3259x over previous
"""Trainium2 Bass kernel for nn_AstroSymbolicEpisodicLayer.

Strategy
--------
8 cores = (batch b in 0..3) x (query-half h in 0..1). One SPMD program; all
per-core differences are data (each core's qT has its own query half first).

Math decomposition (validated to ~1e-6 in fp32 vs the jax reference):
 - Circular-convolution binding via a packed real DFT implemented as
   1024x1024 matmuls.  Packed layout: rows 0..511 are Re bins 0..511, row 512
   is Re bin 512, rows 513..1023 are Im bins 1..511.  Pointwise complex
   multiply pairs row f with row 512+f (tile j with tile j+4), plus a 2-row
   fixup for the purely-real bins 0 and 512.
 - rfft(role_vecs) = role_weights @ rfft(role_norm) by linearity; the softmax
   denominator cancels in the K_bound l2-normalization so we use plain exp.
 - Host weight foldings remove two full projection stages: K is never
   materialized (Kf = kT.T @ (Wk F) + bk F and hidden = relu(kT.T @ (Wk Wr1)
   + bk Wr1 + br1)), and the output projection folds into V (VW = vT.T @
   (Wv Wo); the bv term becomes a constant output bias because the attention
   rows are normalized).
 - Per-token normalizations are folded: 1/||K_bound|| becomes a per-partition
   scalar in the scores epilogue (scores computed keys-major), 1/||Q|| scales
   q_n in SBUF, and the attention row-sum is folded into the output epilogue.

Everything is feature-major ("xT" = [feature, token]) so every chained matmul
is out = lhsT.T @ rhs with contraction on the partition dim.
"""

import sys

for _p in ("/opt/trn_rl_repo", "/root/.axon_site/_ro/trn_rl_repo"):
    if _p not in sys.path:
        sys.path.append(_p)

import numpy as np
import ml_dtypes

import concourse.bass as bass
import concourse.bacc as bacc
import concourse.tile as tile
from concourse import mybir
from concourse.bass_utils import run_bass_kernel_spmd

BF16 = ml_dtypes.bfloat16
F32 = mybir.dt.float32
BF = mybir.dt.bfloat16

B, TQ, TK, D, R, H = 4, 2048, 2048, 1024, 64, 512
TAU_BASE, ASTRO_DECAY = 1.0, 0.95
P = 128
NJ = D // P            # 8 feature chunks
NT = TK // 512         # 4 token chunks of 512
TQH = TQ // 2          # per-core query rows (1024)
NQC = TQH // 512       # 2 query chunks
NKT = TK // P          # 16 key tiles of 128
AF = mybir.ActivationFunctionType
ALU = mybir.AluOpType


# ---------------------------------------------------------------- host consts
def _build_dft_mats(n=1024):
    j = np.arange(n)[:, None].astype(np.float64)
    f = np.arange(n // 2)[None, :].astype(np.float64)
    F = np.zeros((n, n), np.float64)
    F[:, :512] = np.cos(2 * np.pi * j * f / n)
    F[:, 512] = np.cos(np.pi * j[:, 0])
    fi = np.arange(1, 512)[None, :].astype(np.float64)
    F[:, 513:] = -np.sin(2 * np.pi * j * fi / n)

    G = np.zeros((n, n), np.float64)
    d = np.arange(n)[None, :].astype(np.float64)
    G[0, :] = 1.0 / n
    ff = np.arange(1, 512)[:, None].astype(np.float64)
    G[1:512, :] = 2.0 * np.cos(2 * np.pi * ff * d / n) / n
    G[512, :] = ((-1.0) ** d[0]) / n
    G[513:, :] = -2.0 * np.sin(2 * np.pi * ff * d / n) / n
    return F.astype(np.float32), G.astype(np.float32)


def _pack_rfft(x):
    X = np.fft.rfft(np.asarray(x, np.float64), axis=-1)
    out = np.empty(x.shape, np.float32)
    out[..., :512] = X.real[..., :512]
    out[..., 512] = X.real[..., 512]
    out[..., 513:] = X.imag[..., 1:512]
    return out


# ---------------------------------------------------------------- bass kernel
def _emit(nc: bass.Bass):
    # ---- I/O declarations (names = in_map keys) ----
    din = {}
    for nm, shp, dt in [
        ("qT", [D, TQ], BF), ("kT", [D, TK], BF), ("vT", [D, TK], BF),
        ("wq", [D, D], BF), ("wkf", [D, D], BF), ("gmat", [D, D], BF),
        ("vwo", [D, D], BF), ("wkr1", [D, H], BF), ("wr2", [H, R], BF),
        ("pf", [R, D], BF),
        ("bq", [P, NJ], F32), ("bkf", [P, NJ], F32), ("bvo", [P, NJ], F32),
        ("brk", [P, H // P], F32), ("scal", [1, 2], F32),
    ]:
        din[nm] = nc.declare_dram_parameter(nm, shp, dt, isOutput=False)
    outT = nc.declare_dram_parameter("outT", [D, TQH], F32, isOutput=True)
    ns_out = nc.declare_dram_parameter("ns_out", [1, 1], F32, isOutput=True)

    # internal DRAM spill tensors
    Kf_d = nc.dram_tensor("Kf_d", [D, TK], BF)
    Rf_d = nc.dram_tensor("Rf_d", [D, TK], BF)
    Z_d = nc.dram_tensor("Z_d", [D, TK], BF)
    kn_d = nc.dram_tensor("kn_d", [D, TK], BF)
    VW_d = nc.dram_tensor("VW_d", [TK, D], BF)
    rk_stage = nc.dram_tensor("rk_stage", [1, TK], F32)

    c3 = lambda t: t.rearrange("(po pi) x -> pi po x", pi=P)

    with tile.TileContext(nc) as tc, \
         tc.tile_pool(name="w", bufs=2) as wpool, \
         tc.tile_pool(name="wsmall", bufs=1) as wsmall, \
         tc.tile_pool(name="instream", bufs=2) as instream, \
         tc.tile_pool(name="outstage", bufs=4) as outstage, \
         tc.tile_pool(name="persist", bufs=1) as persist, \
         tc.tile_pool(name="cmul", bufs=4) as cmulpool, \
         tc.tile_pool(name="zstage", bufs=2) as zstage, \
         tc.tile_pool(name="scores", bufs=2) as scorepool, \
         tc.tile_pool(name="attst", bufs=3) as attst, \
         tc.tile_pool(name="tmp", bufs=2) as tmp, \
         tc.tile_pool(name="cmtmp", bufs=2) as cmtmp, \
         tc.tile_pool(name="small", bufs=1) as small, \
         tc.tile_pool(name="rows", bufs=2) as rows, \
         tc.tile_pool(name="ps", bufs=4, space="PSUM") as ps, \
         tc.tile_pool(name="psrow", bufs=3, space="PSUM") as psrow:

        # ---- constants ----
        ones_k = small.tile([P, 1], BF, tag="ones_k")      # lhsT for partition sums
        nc.vector.memset(ones_k, 1.0)
        ones_b = small.tile([1, P], F32, tag="ones_b")     # lhsT for row broadcast
        nc.vector.memset(ones_b, 1.0)
        bq_sb = small.tile([P, NJ], F32, tag="bq")
        nc.sync.dma_start(bq_sb, din["bq"][:])
        bkf_sb = small.tile([P, NJ], F32, tag="bkf")
        nc.sync.dma_start(bkf_sb, din["bkf"][:])
        bvo_sb = small.tile([P, NJ], F32, tag="bvo")
        nc.sync.dma_start(bvo_sb, din["bvo"][:])
        brk_sb = small.tile([P, H // P], F32, tag="brk")
        nc.sync.dma_start(brk_sb, din["brk"][:])
        scal_sb = small.tile([1, 2], F32, tag="scal")
        nc.sync.dma_start(scal_sb, din["scal"][:])

        # pre-touch const tiles so later consumers don't accumulate one sync
        # wait per DMA queue on top of their compute waits (ACT has a small
        # hardware wait table)
        pre_s = small.tile([P, 1], F32, tag="pre_s")
        nc.scalar.activation(pre_s, bq_sb[:, 0:1], AF.Identity, bias=bkf_sb[:, 0:1])
        nc.scalar.activation(pre_s, pre_s, AF.Relu, bias=brk_sb[:, 0:1])
        pre_v = small.tile([P, 1], F32, tag="pre_v")
        nc.vector.tensor_scalar_add(pre_v, pre_s, bvo_sb[:, 0:1])
        pre_v2 = small.tile([1, 1], F32, tag="pre_v2")
        nc.vector.tensor_scalar_add(pre_v2, scal_sb[0:1, 0:1], 0.0)

        def load_w(name, shape3):
            t = wpool.tile(shape3, BF, tag="w", name=name)
            nc.sync.dma_start(t, c3(din[name][:]))
            return t

        # ==================== S_AB: Kf = WkF.T @ kT + bkF ; hidden (one pass)
        wkf = load_w("wkf", [P, NJ, D])
        wkr1 = wsmall.tile([P, NJ, H], BF, tag="wkr1")
        nc.sync.dma_start(wkr1, c3(din["wkr1"][:]))
        hidden = scorepool.tile([P, H // P, TK], BF, tag="scT", name="hidden")
        for t in range(NT):
            kin = instream.tile([P, NJ, 512], BF, tag="instream")
            nc.sync.dma_start(kin, c3(din["kT"][:])[:, :, t * 512:(t + 1) * 512])
            for j in range(NJ):
                pst = ps.tile([P, 512], F32, tag="mm")
                for i in range(NJ):
                    nc.tensor.matmul(pst, wkf[:, i, j * P:(j + 1) * P], kin[:, i, :],
                                     start=(i == 0), stop=(i == NJ - 1))
                kf_o = outstage.tile([P, 512], BF, tag="outstage")
                nc.scalar.activation(kf_o, pst, AF.Identity, bias=bkf_sb[:, j:j + 1])
                nc.sync.dma_start(Kf_d[j * P:(j + 1) * P, t * 512:(t + 1) * 512], kf_o)
            for jh in range(H // P):
                pst = ps.tile([P, 512], F32, tag="mm")
                for i in range(NJ):
                    nc.tensor.matmul(pst, wkr1[:, i, jh * P:(jh + 1) * P], kin[:, i, :],
                                     start=(i == 0), stop=(i == NJ - 1))
                nc.scalar.activation(hidden[:, jh, t * 512:(t + 1) * 512], pst,
                                     AF.Relu, bias=brk_sb[:, jh:jh + 1])

        # ==================== S_C: role logits -> exp -> Rf
        wr2 = wsmall.tile([P, H // P, R], BF, tag="wr2")
        nc.sync.dma_start(wr2, c3(din["wr2"][:]))
        rw = persist.tile([R, TK], BF, tag="rw")
        for t in range(NT):
            ps64 = ps.tile([R, 512], F32, tag="mm")
            for i2 in range(H // P):
                nc.tensor.matmul(ps64, wr2[:, i2, :], hidden[:, i2, t * 512:(t + 1) * 512],
                                 start=(i2 == 0), stop=(i2 == H // P - 1))
            nc.scalar.activation(rw[:, t * 512:(t + 1) * 512], ps64, AF.Exp)
        pf = wsmall.tile([R, D], BF, tag="pf")
        nc.sync.dma_start(pf, din["pf"][:])
        for t in range(NT):
            for j in range(NJ):
                pst = ps.tile([P, 512], F32, tag="mm")
                nc.tensor.matmul(pst, pf[:, j * P:(j + 1) * P],
                                 rw[:, t * 512:(t + 1) * 512], start=True, stop=True)
                rf_o = outstage.tile([P, 512], BF, tag="outstage")
                nc.scalar.activation(rf_o, pst, AF.Copy)
                nc.sync.dma_start(Rf_d[j * P:(j + 1) * P, t * 512:(t + 1) * 512], rf_o)

        # ==================== S_D: Z = cmul(Kf, Rf)  (DVE; overlapped by S_E/S_F)
        # packed pairing: row f (tile j) with row 512+f (tile j+4)
        for j in range(4):
            alo = cmulpool.tile([P, TK], BF, tag="cmul")
            nc.sync.dma_start(alo, Kf_d[j * P:(j + 1) * P, :])
            ahi = cmulpool.tile([P, TK], BF, tag="cmul")
            nc.sync.dma_start(ahi, Kf_d[(j + 4) * P:(j + 5) * P, :])
            blo = cmulpool.tile([P, TK], BF, tag="cmul")
            nc.sync.dma_start(blo, Rf_d[j * P:(j + 1) * P, :])
            bhi = cmulpool.tile([P, TK], BF, tag="cmul")
            nc.sync.dma_start(bhi, Rf_d[(j + 4) * P:(j + 5) * P, :])
            t2 = cmtmp.tile([P, TK], BF, tag="cm_t2")
            zlo = zstage.tile([P, TK], BF, tag="zst")
            zhi = zstage.tile([P, TK], BF, tag="zst")
            nc.vector.tensor_mul(zlo, alo, blo)
            nc.vector.tensor_mul(t2, ahi, bhi)
            nc.vector.tensor_tensor(zlo, zlo, t2, ALU.subtract)
            nc.vector.tensor_mul(zhi, alo, bhi)
            nc.vector.tensor_mul(t2, ahi, blo)
            nc.vector.tensor_tensor(zhi, zhi, t2, ALU.add)
            if j == 0:
                # rows 0 (re_0) and 512 (re_512) are purely real
                nc.vector.tensor_mul(zlo[0:1, :], alo[0:1, :], blo[0:1, :])
                nc.vector.tensor_mul(zhi[0:1, :], ahi[0:1, :], bhi[0:1, :])
            nc.sync.dma_start(Z_d[j * P:(j + 1) * P, :], zlo)
            nc.sync.dma_start(Z_d[(j + 4) * P:(j + 5) * P, :], zhi)

        # ==================== S_E: VW = vT.T @ (WvWo)   (fills the cmul hole)
        vwo = load_w("vwo", [P, NJ, D])
        for to in range(NKT):
            vin = attst.tile([P, NJ, P], BF, tag="knstream", name="vin")
            nc.sync.dma_start(vin, c3(din["vT"][:])[:, :, to * P:(to + 1) * P])
            for n in range(2):
                pst = ps.tile([P, 512], F32, tag="mm")
                for i in range(NJ):
                    nc.tensor.matmul(pst, vin[:, i, :], vwo[:, i, n * 512:(n + 1) * 512],
                                     start=(i == 0), stop=(i == NJ - 1))
                v_o = outstage.tile([P, 512], BF, tag="outstage")
                nc.scalar.activation(v_o, pst, AF.Copy)
                nc.sync.dma_start(VW_d[to * P:(to + 1) * P, n * 512:(n + 1) * 512], v_o)

        # ==================== S_F: Q proj (full TQ), tau, q_n
        wq = load_w("wq", [P, NJ, D])
        qn = persist.tile([P, NJ, TQH], BF, tag="qn")
        rq_bc = small.tile([P, TQH], F32, tag="rq_bc")
        ssc = small.tile([1, NT], F32, tag="ssc")
        for t in range(NT):  # over full TQ; chunks 0,1 are this core's half
            qin = instream.tile([P, NJ, 512], BF, tag="instream")
            nc.sync.dma_start(qin, c3(din["qT"][:])[:, :, t * 512:(t + 1) * 512])
            psn = psrow.tile([1, 512], F32, tag="rowsum")
            for j in range(NJ):
                pst = ps.tile([P, 512], F32, tag="mm")
                for i in range(NJ):
                    nc.tensor.matmul(pst, wq[:, i, j * P:(j + 1) * P], qin[:, i, :],
                                     start=(i == 0), stop=(i == NJ - 1))
                if t < NQC:
                    qv = qn[:, j, t * 512:(t + 1) * 512]
                else:
                    qv = tmp.tile([P, 512], BF, tag="qv_other")
                nc.scalar.activation(qv, pst, AF.Identity, bias=bq_sb[:, j:j + 1])
                sq = tmp.tile([P, 512], BF, tag="sq")
                nc.vector.tensor_mul(sq, qv, qv)
                nc.tensor.matmul(psn, ones_k, sq, start=(j == 0), stop=(j == NJ - 1))
            # sqrt of norms; accumulate sum(sqrt) for the surprise mean
            srow = rows.tile([1, 512], F32, tag="srow")
            nc.scalar.activation(srow, psn, AF.Sqrt, accum_out=ssc[0:1, t:t + 1])
            if t < NQC:  # own half: rq broadcast for q_n scaling
                rrow = rows.tile([1, 512], F32, tag="rrow")
                nc.vector.reciprocal(rrow, srow)
                rb_ps = ps.tile([P, 512], F32, tag="mm")
                nc.tensor.matmul(rb_ps, ones_b, rrow, start=True, stop=True)
                nc.vector.tensor_copy(rq_bc[:, t * 512:(t + 1) * 512], rb_ps)
        ss = small.tile([1, 1], F32, tag="ss")
        nc.vector.reduce_sum(ss, ssc, axis=mybir.AxisListType.X)
        # new_state = 0.95*astro + 0.05*ss/(32*2048)
        ns_t = small.tile([1, 1], F32, tag="ns_t")
        v1 = small.tile([1, 1], F32, tag="v1")
        nc.vector.tensor_scalar_mul(v1, ss, (1.0 - ASTRO_DECAY) / (32.0 * TQ))
        v2 = small.tile([1, 1], F32, tag="v2")
        nc.vector.tensor_scalar_mul(v2, scal_sb[0:1, 0:1], ASTRO_DECAY)
        nc.vector.tensor_add(ns_t, v1, v2)
        nc.sync.dma_start(ns_out[:], ns_t)
        # c = tau/4 = max(1 + astro_scale*ns, 0.001)/4
        c_t = small.tile([1, 1], F32, tag="c_t")
        nc.vector.tensor_mul(c_t, ns_t, scal_sb[0:1, 1:2])
        nc.vector.tensor_scalar(c_t, c_t, 1.0, 0.001, ALU.add, ALU.max)
        nc.vector.tensor_scalar_mul(c_t, c_t, 0.25 * TAU_BASE)
        cb_ps = ps.tile([P, 1], F32, tag="mm")
        nc.tensor.matmul(cb_ps, ones_b, c_t, start=True, stop=True)
        c_sb = small.tile([P, 1], F32, tag="c_sb")
        nc.vector.tensor_copy(c_sb, cb_ps)
        for j in range(NJ):
            nc.vector.tensor_mul(qn[:, j, :], qn[:, j, :], rq_bc)

        # ==================== S_G: KB = G.T @ Z, norms -> kn_d, rk
        gmat = load_w("gmat", [P, NJ, D])
        for t in range(NT):
            ztc = instream.tile([P, NJ, 512], BF, tag="instream")
            nc.sync.dma_start(ztc, c3(Z_d[:])[:, :, t * 512:(t + 1) * 512])
            psn = psrow.tile([1, 512], F32, tag="rowsum")
            for j in range(NJ):
                pst = ps.tile([P, 512], F32, tag="mm")
                for i in range(NJ):
                    nc.tensor.matmul(pst, gmat[:, i, j * P:(j + 1) * P], ztc[:, i, :],
                                     start=(i == 0), stop=(i == NJ - 1))
                kb_o = outstage.tile([P, 512], BF, tag="outstage")
                nc.scalar.activation(kb_o, pst, AF.Copy)
                nc.sync.dma_start(kn_d[j * P:(j + 1) * P, t * 512:(t + 1) * 512], kb_o)
                sq = tmp.tile([P, 512], BF, tag="sq")
                nc.vector.tensor_mul(sq, kb_o, kb_o)
                nc.tensor.matmul(psn, ones_k, sq, start=(j == 0), stop=(j == NJ - 1))
            # rk chunk = 1/sqrt(norm2) -> stage to DRAM for the column reload
            srow = rows.tile([1, 512], F32, tag="srow")
            nc.scalar.activation(srow, psn, AF.Sqrt)
            rrow = rows.tile([1, 512], F32, tag="rrow")
            nc.vector.reciprocal(rrow, srow)
            nc.sync.dma_start(rk_stage[0:1, t * 512:(t + 1) * 512], rrow)
        # rk as [128,16] per-key-tile partition scalars, negated for the
        # (S*(-rk) + 1) epilogue
        rk_col = small.tile([P, NKT], F32, tag="rk_col")
        nc.sync.dma_start(
            rk_col, rk_stage.rearrange("o (kt p) -> (o p) kt", p=P))
        nrk_col = small.tile([P, NKT], F32, tag="nrk_col")
        nc.vector.tensor_scalar_mul(nrk_col, rk_col, -1.0)

        # ==================== S_H: attention scores (kt outer, both qc)
        scT = [scorepool.tile([P, NKT, 512], BF, tag="scT", name=f"scT{qc}")
               for qc in range(NQC)]
        rs_ps = [psrow.tile([1, 512], F32, tag="rowsum", name=f"rs_ps{qc}")
                 for qc in range(NQC)]
        for kt in range(NKT):
            knc = attst.tile([P, NJ, P], BF, tag="knstream", name="knc")
            nc.sync.dma_start(knc, c3(kn_d[:])[:, :, kt * P:(kt + 1) * P])
            for qc in range(NQC):
                ps_s = ps.tile([P, 512], F32, tag="mm")
                for j in range(NJ):
                    nc.tensor.matmul(ps_s, knc[:, j, :], qn[:, j, qc * 512:(qc + 1) * 512],
                                     start=(j == 0), stop=(j == NJ - 1))
                u0 = tmp.tile([P, 512], BF, tag="u0")
                # u0 = 1 - cos = S*(-rk[key]) + 1
                nc.vector.tensor_scalar(u0, ps_s, nrk_col[:, kt:kt + 1], 1.0,
                                        ALU.mult, ALU.add)
                u1 = tmp.tile([P, 512], BF, tag="u1")
                nc.scalar.activation(u1, u0, AF.Square)
                u2 = tmp.tile([P, 512], BF, tag="u2")
                nc.vector.tensor_scalar_mul(u2, u1, c_sb)
                # scores = relu(1 - c*(1-cos)^2)
                nc.scalar.activation(scT[qc][:, kt, :], u2, AF.Relu,
                                     bias=1.0, scale=-1.0)
                nc.tensor.matmul(rs_ps[qc], ones_k, scT[qc][:, kt, :],
                                 start=(kt == 0), stop=(kt == NKT - 1))
        # rowsum reciprocal -> broadcast [128,512] per qc
        rs_bc = []
        for qc in range(NQC):
            rs_row = small.tile([1, 512], F32, tag=f"rs_row{qc}", name=f"rs_row{qc}")
            nc.vector.reciprocal(rs_row, rs_ps[qc])
            rb_ps = ps.tile([P, 512], F32, tag="mm")
            nc.tensor.matmul(rb_ps, ones_b, rs_row, start=True, stop=True)
            rb = small.tile([P, 512], F32, tag=f"rs_bc{qc}", name=f"rs_bc{qc}")
            nc.vector.tensor_copy(rb, rb_ps)
            rs_bc.append(rb)

        # ==================== S_I: out = (scT.T @ VW)*rs + bvo  (Wo folded in)
        for qc in range(NQC):
            for j2g in range(2):  # Dout groups of 512 (4 psum tiles each)
                pcs = [ps.tile([P, 512], F32, tag="mm", name=f"pcs{i}") for i in range(4)]
                for kt in range(NKT):
                    vkt = attst.tile([P, 512], BF, tag="vstream8", name="vkt")
                    nc.sync.dma_start(
                        vkt, VW_d[kt * P:(kt + 1) * P, j2g * 512:(j2g + 1) * 512])
                    for jj in range(4):
                        nc.tensor.matmul(pcs[jj], vkt[:, jj * P:(jj + 1) * P],
                                         scT[qc][:, kt, :],
                                         start=(kt == 0), stop=(kt == NKT - 1))
                for jj in range(4):
                    j = j2g * 4 + jj
                    o_t = tmp.tile([P, 512], F32, tag="o_t")
                    nc.vector.tensor_mul(o_t, pcs[jj], rs_bc[qc])
                    nc.vector.tensor_scalar_add(o_t, o_t, bvo_sb[:, j:j + 1])
                    nc.sync.dma_start(
                        outT[j * P:(j + 1) * P, qc * 512:(qc + 1) * 512], o_t)

    return nc


_CACHE = {}


def _get_nc():
    if "nc" not in _CACHE:
        nc = bacc.Bacc(None, target_bir_lowering=False)
        _emit(nc)
        nc.finalize()
        _CACHE["nc"] = nc
    return _CACHE["nc"]


def build_in_maps(inputs):
    """Host-side prep: foldings, packing, per-core sharding."""
    F, G = _build_dft_mats(D)
    role = np.asarray(inputs["role_matrix"], np.float32)
    role = role / np.clip(np.linalg.norm(role, axis=-1, keepdims=True), 1e-12, None)
    PF = _pack_rfft(role)

    f32 = lambda x: np.asarray(x, np.float32)
    bf = lambda x: np.ascontiguousarray(f32(x)).astype(BF16)
    btile = lambda x: np.ascontiguousarray(f32(x).reshape(-1, P).T.copy())

    Wk, Wv, Wo, Wr1 = f32(inputs["Wk"]), f32(inputs["Wv"]), f32(inputs["Wo"]), f32(inputs["Wr1"])
    bk, bv, bo, br1 = f32(inputs["bk"]), f32(inputs["bv"]), f32(inputs["bo"]), f32(inputs["br1"])

    weights = {
        "wq": bf(inputs["Wq"]),
        "wkf": bf(Wk @ F),
        "gmat": bf(G),
        "vwo": bf(Wv @ Wo),
        "wkr1": bf(Wk @ Wr1),
        "wr2": bf(inputs["Wr2"]),
        "pf": bf(PF),
        "bq": btile(inputs["bq"]),
        "bkf": btile(bk @ F),
        "bvo": btile(bv @ Wo + bo),
        "brk": btile(bk @ Wr1 + br1),
    }

    in_maps = []
    for core in range(8):
        b, h = core // 2, core % 2
        q = f32(inputs["q_in"][b])
        own = q[h * TQH:(h + 1) * TQH]
        other = q[(1 - h) * TQH:(2 - h) * TQH]
        m = dict(weights)
        m["qT"] = np.ascontiguousarray(np.concatenate([own, other], 0).T).astype(BF16)
        m["kT"] = np.ascontiguousarray(f32(inputs["k_in"][b]).T).astype(BF16)
        m["vT"] = np.ascontiguousarray(f32(inputs["v_in"][b]).T).astype(BF16)
        m["scal"] = np.array(
            [[np.float32(inputs["astrocyte_state"][b]),
              np.float32(np.asarray(inputs["astro_scale"]).reshape(-1)[0])]],
            np.float32)
        in_maps.append(m)
    return in_maps


def kernel(q_in, k_in, v_in, astrocyte_state, Wq, bq, Wk, bk, Wv, bv, Wo, bo,
           role_matrix, Wr1, br1, Wr2, astro_scale, **_ignored):
    nc = _get_nc()
    inputs = dict(q_in=q_in, k_in=k_in, v_in=v_in, astrocyte_state=astrocyte_state,
                  Wq=Wq, bq=bq, Wk=Wk, bk=bk, Wv=Wv, bv=bv, Wo=Wo, bo=bo,
                  role_matrix=role_matrix, Wr1=Wr1, br1=br1, Wr2=Wr2,
                  astro_scale=astro_scale)
    in_maps = build_in_maps(inputs)
    res = run_bass_kernel_spmd(nc, in_maps, core_ids=list(range(8)))

    output = np.empty((B, TQ, D), np.float32)
    new_state = np.empty((B,), np.float32)
    for core in range(8):
        b, h = core // 2, core % 2
        output[b, h * TQH:(h + 1) * TQH, :] = res.results[core]["outT"].T
        if h == 0:
            new_state[b] = res.results[core]["ns_out"][0, 0]
    return output, new_state


# revision 21
# speedup vs baseline: 1.3577x; 1.0240x over previous
"""Trainium2 Bass kernel for nn_AstroSymbolicEpisodicLayer.

Strategy
--------
8 cores = (batch b in 0..3) x (query-half h in 0..1). One SPMD program; all
per-core differences are data (each core's qT has its own query half first).

Math decomposition (validated to ~1e-6 in fp32 vs the jax reference):
 - Circular-convolution binding via a packed real DFT implemented as
   1024x1024 matmuls.  Packed layout: rows 0..511 are Re bins 0..511, row 512
   is Re bin 512, rows 513..1023 are Im bins 1..511.  Pointwise complex
   multiply pairs row f with row 512+f (tile j with tile j+4), plus a 2-row
   fixup for the purely-real bins 0 and 512.
 - rfft(role_vecs) = role_weights @ rfft(role_norm) by linearity; the softmax
   denominator cancels in the K_bound l2-normalization so we use plain exp.
 - Host weight foldings remove two full projection stages: K is never
   materialized (Kf = kT.T @ (Wk F) + bk F and hidden = relu(kT.T @ (Wk Wr1)
   + bk Wr1 + br1)), and the output projection folds into V (VW = vT.T @
   (Wv Wo); the bv term becomes a constant output bias because the attention
   rows are normalized).
 - Per-token normalizations are folded: 1/||K_bound|| becomes a per-partition
   scalar in the scores epilogue (scores computed keys-major), 1/||Q|| scales
   q_n in SBUF, and the attention row-sum is folded into the output epilogue.

Everything is feature-major ("xT" = [feature, token]) so every chained matmul
is out = lhsT.T @ rhs with contraction on the partition dim.
"""

import sys

for _p in ("/opt/trn_rl_repo", "/root/.axon_site/_ro/trn_rl_repo"):
    if _p not in sys.path:
        sys.path.append(_p)

import numpy as np
import ml_dtypes

import concourse.bass as bass
import concourse.bacc as bacc
import concourse.tile as tile
from concourse import mybir
from concourse.bass_utils import run_bass_kernel_spmd

BF16 = ml_dtypes.bfloat16
F32 = mybir.dt.float32
BF = mybir.dt.bfloat16

B, TQ, TK, D, R, H = 4, 2048, 2048, 1024, 64, 512
TAU_BASE, ASTRO_DECAY = 1.0, 0.95
P = 128
NJ = D // P            # 8 feature chunks
NT = TK // 512         # 4 token chunks of 512
TQH = TQ // 2          # per-core query rows (1024)
NQC = TQH // 512       # 2 query chunks
NKT = TK // P          # 16 key tiles of 128
AF = mybir.ActivationFunctionType
ALU = mybir.AluOpType


# ---------------------------------------------------------------- host consts
def _build_dft_mats(n=1024):
    j = np.arange(n)[:, None].astype(np.float64)
    f = np.arange(n // 2)[None, :].astype(np.float64)
    F = np.zeros((n, n), np.float64)
    F[:, :512] = np.cos(2 * np.pi * j * f / n)
    F[:, 512] = np.cos(np.pi * j[:, 0])
    fi = np.arange(1, 512)[None, :].astype(np.float64)
    F[:, 513:] = -np.sin(2 * np.pi * j * fi / n)

    G = np.zeros((n, n), np.float64)
    d = np.arange(n)[None, :].astype(np.float64)
    G[0, :] = 1.0 / n
    ff = np.arange(1, 512)[:, None].astype(np.float64)
    G[1:512, :] = 2.0 * np.cos(2 * np.pi * ff * d / n) / n
    G[512, :] = ((-1.0) ** d[0]) / n
    G[513:, :] = -2.0 * np.sin(2 * np.pi * ff * d / n) / n
    return F.astype(np.float32), G.astype(np.float32)


def _pack_rfft(x):
    X = np.fft.rfft(np.asarray(x, np.float64), axis=-1)
    out = np.empty(x.shape, np.float32)
    out[..., :512] = X.real[..., :512]
    out[..., 512] = X.real[..., 512]
    out[..., 513:] = X.imag[..., 1:512]
    return out


# ---------------------------------------------------------------- bass kernel
def _emit(nc: bass.Bass):
    # ---- I/O declarations (names = in_map keys) ----
    din = {}
    for nm, shp, dt in [
        ("qT", [D, TQ], BF), ("kT", [D, TK], BF), ("vT", [D, TK], BF),
        ("wq", [D, D], BF), ("wkf", [D, D], BF), ("gmat", [D, D], BF),
        ("vwo", [D, D], BF), ("wkr1", [D, H], BF), ("wr2", [H, R], BF),
        ("pf", [R, D], BF),
        ("bq", [P, NJ], F32), ("bkf", [P, NJ], F32), ("bvo", [P, NJ], F32),
        ("brk", [P, H // P], F32), ("scal", [1, 2], F32),
    ]:
        din[nm] = nc.declare_dram_parameter(nm, shp, dt, isOutput=False)
    outT = nc.declare_dram_parameter("outT", [D, TQH], F32, isOutput=True)
    ns_out = nc.declare_dram_parameter("ns_out", [1, 1], F32, isOutput=True)

    # internal DRAM spill tensors
    kn_d = nc.dram_tensor("kn_d", [D, TK], BF)
    VW_d = nc.dram_tensor("VW_d", [TK, D], BF)
    rk_stage = nc.dram_tensor("rk_stage", [1, TK], F32)

    c3 = lambda t: t.rearrange("(po pi) x -> pi po x", pi=P)

    with tile.TileContext(nc) as tc, \
         tc.tile_pool(name="w", bufs=2) as wpool, \
         tc.tile_pool(name="wsmall", bufs=1) as wsmall, \
         tc.tile_pool(name="instream", bufs=2) as instream, \
         tc.tile_pool(name="outstage", bufs=3) as outstage, \
         tc.tile_pool(name="persist", bufs=1) as persist, \
         tc.tile_pool(name="kfc", bufs=2) as kfcpool, \
         tc.tile_pool(name="scores", bufs=2) as scorepool, \
         tc.tile_pool(name="attst", bufs=2) as attst, \
         tc.tile_pool(name="tmp", bufs=2) as tmp, \
         tc.tile_pool(name="cmt", bufs=2) as cmt, \
         tc.tile_pool(name="small", bufs=1) as small, \
         tc.tile_pool(name="rows", bufs=1) as rows, \
         tc.tile_pool(name="ps", bufs=4, space="PSUM") as ps, \
         tc.tile_pool(name="psrow", bufs=3, space="PSUM") as psrow:

        # ---- constants ----
        ones_k = small.tile([P, 1], BF, tag="ones_k")      # lhsT for partition sums
        nc.vector.memset(ones_k, 1.0)
        ones_b = small.tile([1, P], F32, tag="ones_b")     # lhsT for row broadcast
        nc.vector.memset(ones_b, 1.0)
        bq_sb = small.tile([P, NJ], F32, tag="bq")
        nc.sync.dma_start(bq_sb, din["bq"][:])
        bkf_sb = small.tile([P, NJ], F32, tag="bkf")
        nc.sync.dma_start(bkf_sb, din["bkf"][:])
        bvo_sb = small.tile([P, NJ], F32, tag="bvo")
        nc.sync.dma_start(bvo_sb, din["bvo"][:])
        brk_sb = small.tile([P, H // P], F32, tag="brk")
        nc.sync.dma_start(brk_sb, din["brk"][:])
        scal_sb = small.tile([1, 2], F32, tag="scal")
        nc.sync.dma_start(scal_sb, din["scal"][:])

        # pre-touch const tiles so later consumers don't accumulate one sync
        # wait per DMA queue on top of their compute waits (ACT has a small
        # hardware wait table)
        pre_s = small.tile([P, 1], F32, tag="pre_s")
        nc.scalar.activation(pre_s, bq_sb[:, 0:1], AF.Identity, bias=bkf_sb[:, 0:1])
        nc.scalar.activation(pre_s, pre_s, AF.Relu, bias=brk_sb[:, 0:1])
        pre_v = small.tile([P, 1], F32, tag="pre_v")
        nc.vector.tensor_scalar_add(pre_v, pre_s, bvo_sb[:, 0:1])
        pre_v2 = small.tile([1, 1], F32, tag="pre_v2")
        nc.vector.tensor_scalar_add(pre_v2, scal_sb[0:1, 0:1], 0.0)

        def load_w(name, shape3):
            t = wpool.tile(shape3, BF, tag="w", name=name)
            nc.sync.dma_start(t, c3(din[name][:]))
            return t

        # ==================== S_A: hidden = relu(WkR1.T @ kT + brk)
        wkr1 = wsmall.tile([P, NJ, H], BF, tag="wkr1")
        nc.sync.dma_start(wkr1, c3(din["wkr1"][:]))
        hidden = scorepool.tile([P, H // P, TK], BF, tag="scT", name="hidden")
        for t in range(NT):
            kin = instream.tile([P, NJ, 512], BF, tag="instream")
            nc.sync.dma_start(kin, c3(din["kT"][:])[:, :, t * 512:(t + 1) * 512])
            for jh in range(H // P):
                pst = ps.tile([P, 512], F32, tag="mm")
                for i in range(NJ):
                    nc.tensor.matmul(pst, wkr1[:, i, jh * P:(jh + 1) * P], kin[:, i, :],
                                     start=(i == 0), stop=(i == NJ - 1))
                nc.scalar.activation(hidden[:, jh, t * 512:(t + 1) * 512], pst,
                                     AF.Relu, bias=brk_sb[:, jh:jh + 1])

        # ==================== S_C: role logits -> exp -> Rf (SBUF resident)
        wr2 = wsmall.tile([P, H // P, R], BF, tag="wr2")
        nc.sync.dma_start(wr2, c3(din["wr2"][:]))
        rw = persist.tile([R, TK], BF, tag="rw")
        for t in range(NT):
            ps64 = ps.tile([R, 512], F32, tag="mm")
            for i2 in range(H // P):
                nc.tensor.matmul(ps64, wr2[:, i2, :], hidden[:, i2, t * 512:(t + 1) * 512],
                                 start=(i2 == 0), stop=(i2 == H // P - 1))
            nc.scalar.activation(rw[:, t * 512:(t + 1) * 512], ps64, AF.Exp)
        pf = wsmall.tile([R, D], BF, tag="pf")
        nc.sync.dma_start(pf, din["pf"][:])
        Rf = persist.tile([P, NJ, TK], BF, tag="Rf")
        for t in range(NT):
            for j in range(NJ):
                pst = ps.tile([P, 512], F32, tag="mm")
                nc.tensor.matmul(pst, pf[:, j * P:(j + 1) * P],
                                 rw[:, t * 512:(t + 1) * 512], start=True, stop=True)
                nc.scalar.activation(Rf[:, j, t * 512:(t + 1) * 512], pst, AF.Copy)

        # ==================== S_B: fused Kf -> cmul -> KB per token chunk
        # Kf(t) matmuls, then (interleaved one chunk later so the DVE cmul
        # hides behind PE work) KB(t) = G.T @ Z(t) with Z computed in-place
        # in the kf chunk.  Packed pairing: tile j with tile j+4.
        wkf = load_w("wkf", [P, NJ, D])
        gmat = load_w("gmat", [P, NJ, D])

        kfc = [None] * NT

        def emit_kf(t):
            kin = instream.tile([P, NJ, 512], BF, tag="instream", name=f"kin2_{t}")
            nc.sync.dma_start(kin, c3(din["kT"][:])[:, :, t * 512:(t + 1) * 512])
            kfc[t] = kfcpool.tile([P, NJ, 512], BF, tag="kfc", name=f"kfc{t}")
            for j in range(NJ):
                pst = ps.tile([P, 512], F32, tag="mm")
                for i in range(NJ):
                    nc.tensor.matmul(pst, wkf[:, i, j * P:(j + 1) * P], kin[:, i, :],
                                     start=(i == 0), stop=(i == NJ - 1))
                nc.scalar.activation(kfc[t][:, j, :], pst, AF.Identity,
                                     bias=bkf_sb[:, j:j + 1])
            # cmul in place: kf chunk becomes Z chunk (DVE)
            kf = kfc[t]
            rfs = Rf[:, :, t * 512:(t + 1) * 512]
            zf0 = cmt.tile([1, 512], BF, tag="zf0")
            zf4 = cmt.tile([1, 512], BF, tag="zf4")
            nc.vector.tensor_mul(zf0, kf[0:1, 0, :], rfs[0:1, 0, :])
            nc.vector.tensor_mul(zf4, kf[0:1, 4, :], rfs[0:1, 4, :])
            for j in range(4):
                t2 = cmt.tile([P, 512], BF, tag="cm_t2")
                t3 = cmt.tile([P, 512], BF, tag="cm_t3")
                nc.vector.tensor_mul(t2, kf[:, j + 4, :], rfs[:, j + 4, :])
                nc.vector.tensor_mul(t3, kf[:, j + 4, :], rfs[:, j, :])
                nc.vector.tensor_mul(kf[:, j + 4, :], kf[:, j, :], rfs[:, j + 4, :])
                nc.vector.tensor_tensor(kf[:, j + 4, :], kf[:, j + 4, :], t3, ALU.add)
                nc.vector.tensor_mul(kf[:, j, :], kf[:, j, :], rfs[:, j, :])
                nc.vector.tensor_tensor(kf[:, j, :], kf[:, j, :], t2, ALU.subtract)
            # rows 0 (re_0) and 512 (re_512) are purely real
            nc.vector.tensor_copy(kf[0:1, 0, :], zf0)
            nc.vector.tensor_copy(kf[0:1, 4, :], zf4)

        def emit_kb(t):
            psn = psrow.tile([1, 512], F32, tag="rowsum", name=f"psn_kb{t}")
            for j in range(NJ):
                pst = ps.tile([P, 512], F32, tag="mm")
                for i in range(NJ):
                    nc.tensor.matmul(pst, gmat[:, i, j * P:(j + 1) * P], kfc[t][:, i, :],
                                     start=(i == 0), stop=(i == NJ - 1))
                kb_o = outstage.tile([P, 512], BF, tag="outstage")
                nc.scalar.activation(kb_o, pst, AF.Copy)
                nc.sync.dma_start(kn_d[j * P:(j + 1) * P, t * 512:(t + 1) * 512], kb_o)
                sq = tmp.tile([P, 512], BF, tag="sq")
                nc.vector.tensor_mul(sq, kb_o, kb_o)
                nc.tensor.matmul(psn, ones_k, sq, start=(j == 0), stop=(j == NJ - 1))
            # rk chunk = 1/sqrt(norm2) -> stage to DRAM for the column reload
            srow = rows.tile([1, 512], F32, tag="srow")
            nc.scalar.activation(srow, psn, AF.Sqrt)
            rrow = rows.tile([1, 512], F32, tag="rrow")
            nc.vector.reciprocal(rrow, srow)
            nc.sync.dma_start(rk_stage[0:1, t * 512:(t + 1) * 512], rrow)

        emit_kf(0)
        emit_kf(1)
        emit_kb(0)
        emit_kf(2)
        emit_kb(1)
        emit_kf(3)
        emit_kb(2)
        emit_kb(3)
        # rk as [128,16] per-key-tile partition scalars, negated for the
        # (S*(-rk) + 1) epilogue
        rk_col = small.tile([P, NKT], F32, tag="rk_col")
        nc.sync.dma_start(
            rk_col, rk_stage.rearrange("o (kt p) -> (o p) kt", p=P))
        nrk_col = small.tile([P, NKT], F32, tag="nrk_col")
        nc.vector.tensor_scalar_mul(nrk_col, rk_col, -1.0)

        # ==================== S_E: VW = vT.T @ (WvWo)
        vwo = load_w("vwo", [P, NJ, D])
        for to in range(NKT):
            vin = attst.tile([P, NJ, P], BF, tag="knstream", name="vin")
            nc.sync.dma_start(vin, c3(din["vT"][:])[:, :, to * P:(to + 1) * P])
            for n in range(2):
                pst = ps.tile([P, 512], F32, tag="mm")
                for i in range(NJ):
                    nc.tensor.matmul(pst, vin[:, i, :], vwo[:, i, n * 512:(n + 1) * 512],
                                     start=(i == 0), stop=(i == NJ - 1))
                v_o = outstage.tile([P, 512], BF, tag="outstage")
                nc.scalar.activation(v_o, pst, AF.Copy)
                nc.sync.dma_start(VW_d[to * P:(to + 1) * P, n * 512:(n + 1) * 512], v_o)

        # ==================== S_F: Q proj (full TQ), tau, q_n
        wq = load_w("wq", [P, NJ, D])
        qn = persist.tile([P, NJ, TQH], BF, tag="qn")
        rq_bc = small.tile([P, TQH], F32, tag="rq_bc")
        ssc = small.tile([1, NT], F32, tag="ssc")
        for t in range(NT):  # over full TQ; chunks 0,1 are this core's half
            qin = instream.tile([P, NJ, 512], BF, tag="instream")
            nc.sync.dma_start(qin, c3(din["qT"][:])[:, :, t * 512:(t + 1) * 512])
            psn = psrow.tile([1, 512], F32, tag="rowsum")
            for j in range(NJ):
                pst = ps.tile([P, 512], F32, tag="mm")
                for i in range(NJ):
                    nc.tensor.matmul(pst, wq[:, i, j * P:(j + 1) * P], qin[:, i, :],
                                     start=(i == 0), stop=(i == NJ - 1))
                if t < NQC:
                    qv = qn[:, j, t * 512:(t + 1) * 512]
                else:
                    qv = tmp.tile([P, 512], BF, tag="qv_other")
                nc.scalar.activation(qv, pst, AF.Identity, bias=bq_sb[:, j:j + 1])
                sq = tmp.tile([P, 512], BF, tag="sq")
                nc.vector.tensor_mul(sq, qv, qv)
                nc.tensor.matmul(psn, ones_k, sq, start=(j == 0), stop=(j == NJ - 1))
            # sqrt of norms; accumulate sum(sqrt) for the surprise mean
            srow = rows.tile([1, 512], F32, tag="srow")
            nc.scalar.activation(srow, psn, AF.Sqrt, accum_out=ssc[0:1, t:t + 1])
            if t < NQC:  # own half: rq broadcast for q_n scaling
                rrow = rows.tile([1, 512], F32, tag="rrow")
                nc.vector.reciprocal(rrow, srow)
                rb_ps = ps.tile([P, 512], F32, tag="mm")
                nc.tensor.matmul(rb_ps, ones_b, rrow, start=True, stop=True)
                nc.vector.tensor_copy(rq_bc[:, t * 512:(t + 1) * 512], rb_ps)
        ss = small.tile([1, 1], F32, tag="ss")
        nc.vector.reduce_sum(ss, ssc, axis=mybir.AxisListType.X)
        # new_state = 0.95*astro + 0.05*ss/(32*2048)
        ns_t = small.tile([1, 1], F32, tag="ns_t")
        v1 = small.tile([1, 1], F32, tag="v1")
        nc.vector.tensor_scalar_mul(v1, ss, (1.0 - ASTRO_DECAY) / (32.0 * TQ))
        v2 = small.tile([1, 1], F32, tag="v2")
        nc.vector.tensor_scalar_mul(v2, scal_sb[0:1, 0:1], ASTRO_DECAY)
        nc.vector.tensor_add(ns_t, v1, v2)
        nc.sync.dma_start(ns_out[:], ns_t)
        # c = tau/4 = max(1 + astro_scale*ns, 0.001)/4
        c_t = small.tile([1, 1], F32, tag="c_t")
        nc.vector.tensor_mul(c_t, ns_t, scal_sb[0:1, 1:2])
        nc.vector.tensor_scalar(c_t, c_t, 1.0, 0.001, ALU.add, ALU.max)
        nc.vector.tensor_scalar_mul(c_t, c_t, 0.25 * TAU_BASE)
        cb_ps = ps.tile([P, 1], F32, tag="mm")
        nc.tensor.matmul(cb_ps, ones_b, c_t, start=True, stop=True)
        c_sb = small.tile([P, 1], F32, tag="c_sb")
        nc.vector.tensor_copy(c_sb, cb_ps)
        for j in range(NJ):
            nc.vector.tensor_mul(qn[:, j, :], qn[:, j, :], rq_bc)

        # ==================== S_H: attention scores (kt outer, both qc)
        scT = [scorepool.tile([P, NKT, 512], BF, tag="scT", name=f"scT{qc}")
               for qc in range(NQC)]
        rs_ps = [psrow.tile([1, 512], F32, tag="rowsum", name=f"rs_ps{qc}")
                 for qc in range(NQC)]
        for kt in range(NKT):
            knc = attst.tile([P, NJ, P], BF, tag="knstream", name="knc")
            nc.sync.dma_start(knc, c3(kn_d[:])[:, :, kt * P:(kt + 1) * P])
            for qc in range(NQC):
                ps_s = ps.tile([P, 512], F32, tag="mm")
                for j in range(NJ):
                    nc.tensor.matmul(ps_s, knc[:, j, :], qn[:, j, qc * 512:(qc + 1) * 512],
                                     start=(j == 0), stop=(j == NJ - 1))
                u0 = tmp.tile([P, 512], BF, tag="u0")
                # u0 = 1 - cos = S*(-rk[key]) + 1
                nc.vector.tensor_scalar(u0, ps_s, nrk_col[:, kt:kt + 1], 1.0,
                                        ALU.mult, ALU.add)
                u1 = tmp.tile([P, 512], BF, tag="u1")
                nc.scalar.activation(u1, u0, AF.Square)
                u2 = tmp.tile([P, 512], BF, tag="u2")
                nc.vector.tensor_scalar_mul(u2, u1, c_sb)
                # scores = relu(1 - c*(1-cos)^2)
                nc.scalar.activation(scT[qc][:, kt, :], u2, AF.Relu,
                                     bias=1.0, scale=-1.0)
                nc.tensor.matmul(rs_ps[qc], ones_k, scT[qc][:, kt, :],
                                 start=(kt == 0), stop=(kt == NKT - 1))
        # rowsum reciprocal -> broadcast [128,512] per qc
        rs_bc = []
        for qc in range(NQC):
            rs_row = small.tile([1, 512], F32, tag=f"rs_row{qc}", name=f"rs_row{qc}")
            nc.vector.reciprocal(rs_row, rs_ps[qc])
            rb_ps = ps.tile([P, 512], F32, tag="mm")
            nc.tensor.matmul(rb_ps, ones_b, rs_row, start=True, stop=True)
            rb = small.tile([P, 512], F32, tag=f"rs_bc{qc}", name=f"rs_bc{qc}")
            nc.vector.tensor_copy(rb, rb_ps)
            rs_bc.append(rb)

        # ==================== S_I: out = (scT.T @ VW)*rs + bvo  (Wo folded in)
        for qc in range(NQC):
            for j2g in range(2):  # Dout groups of 512 (4 psum tiles each)
                pcs = [ps.tile([P, 512], F32, tag="mm", name=f"pcs{i}") for i in range(4)]
                for kt in range(NKT):
                    vkt = attst.tile([P, 512], BF, tag="vstream8", name="vkt")
                    nc.sync.dma_start(
                        vkt, VW_d[kt * P:(kt + 1) * P, j2g * 512:(j2g + 1) * 512])
                    for jj in range(4):
                        nc.tensor.matmul(pcs[jj], vkt[:, jj * P:(jj + 1) * P],
                                         scT[qc][:, kt, :],
                                         start=(kt == 0), stop=(kt == NKT - 1))
                for jj in range(4):
                    j = j2g * 4 + jj
                    o_t = tmp.tile([P, 512], F32, tag="o_t")
                    nc.vector.tensor_mul(o_t, pcs[jj], rs_bc[qc])
                    nc.vector.tensor_scalar_add(o_t, o_t, bvo_sb[:, j:j + 1])
                    nc.sync.dma_start(
                        outT[j * P:(j + 1) * P, qc * 512:(qc + 1) * 512], o_t)

    return nc


_CACHE = {}


def _get_nc():
    if "nc" not in _CACHE:
        nc = bacc.Bacc(None, target_bir_lowering=False)
        _emit(nc)
        nc.finalize()
        _CACHE["nc"] = nc
    return _CACHE["nc"]


def build_in_maps(inputs):
    """Host-side prep: foldings, packing, per-core sharding."""
    F, G = _build_dft_mats(D)
    role = np.asarray(inputs["role_matrix"], np.float32)
    role = role / np.clip(np.linalg.norm(role, axis=-1, keepdims=True), 1e-12, None)
    PF = _pack_rfft(role)

    f32 = lambda x: np.asarray(x, np.float32)
    bf = lambda x: np.ascontiguousarray(f32(x)).astype(BF16)
    btile = lambda x: np.ascontiguousarray(f32(x).reshape(-1, P).T.copy())

    Wk, Wv, Wo, Wr1 = f32(inputs["Wk"]), f32(inputs["Wv"]), f32(inputs["Wo"]), f32(inputs["Wr1"])
    bk, bv, bo, br1 = f32(inputs["bk"]), f32(inputs["bv"]), f32(inputs["bo"]), f32(inputs["br1"])

    weights = {
        "wq": bf(inputs["Wq"]),
        "wkf": bf(Wk @ F),
        "gmat": bf(G),
        "vwo": bf(Wv @ Wo),
        "wkr1": bf(Wk @ Wr1),
        "wr2": bf(inputs["Wr2"]),
        "pf": bf(PF),
        "bq": btile(inputs["bq"]),
        "bkf": btile(bk @ F),
        "bvo": btile(bv @ Wo + bo),
        "brk": btile(bk @ Wr1 + br1),
    }

    in_maps = []
    for core in range(8):
        b, h = core // 2, core % 2
        q = f32(inputs["q_in"][b])
        own = q[h * TQH:(h + 1) * TQH]
        other = q[(1 - h) * TQH:(2 - h) * TQH]
        m = dict(weights)
        m["qT"] = np.ascontiguousarray(np.concatenate([own, other], 0).T).astype(BF16)
        m["kT"] = np.ascontiguousarray(f32(inputs["k_in"][b]).T).astype(BF16)
        m["vT"] = np.ascontiguousarray(f32(inputs["v_in"][b]).T).astype(BF16)
        m["scal"] = np.array(
            [[np.float32(inputs["astrocyte_state"][b]),
              np.float32(np.asarray(inputs["astro_scale"]).reshape(-1)[0])]],
            np.float32)
        in_maps.append(m)
    return in_maps


def kernel(q_in, k_in, v_in, astrocyte_state, Wq, bq, Wk, bk, Wv, bv, Wo, bo,
           role_matrix, Wr1, br1, Wr2, astro_scale, **_ignored):
    nc = _get_nc()
    inputs = dict(q_in=q_in, k_in=k_in, v_in=v_in, astrocyte_state=astrocyte_state,
                  Wq=Wq, bq=bq, Wk=Wk, bk=bk, Wv=Wv, bv=bv, Wo=Wo, bo=bo,
                  role_matrix=role_matrix, Wr1=Wr1, br1=br1, Wr2=Wr2,
                  astro_scale=astro_scale)
    in_maps = build_in_maps(inputs)
    res = run_bass_kernel_spmd(nc, in_maps, core_ids=list(range(8)))

    output = np.empty((B, TQ, D), np.float32)
    new_state = np.empty((B,), np.float32)
    for core in range(8):
        b, h = core // 2, core % 2
        output[b, h * TQH:(h + 1) * TQH, :] = res.results[core]["outT"].T
        if h == 0:
            new_state[b] = res.results[core]["ns_out"][0, 0]
    return output, new_state


# revision 22
# speedup vs baseline: 1.3900x; 1.0238x over previous
"""Trainium2 Bass kernel for nn_AstroSymbolicEpisodicLayer.

Strategy
--------
8 cores = (batch b in 0..3) x (query-half h in 0..1). One SPMD program; all
per-core differences are data (each core's qT has its own query half first).

Math decomposition (validated to ~1e-6 in fp32 vs the jax reference):
 - Circular-convolution binding via a packed real DFT implemented as
   1024x1024 matmuls.  Packed layout: rows 0..511 are Re bins 0..511, row 512
   is Re bin 512, rows 513..1023 are Im bins 1..511.  Pointwise complex
   multiply pairs row f with row 512+f (tile j with tile j+4), plus a 2-row
   fixup for the purely-real bins 0 and 512.
 - rfft(role_vecs) = role_weights @ rfft(role_norm) by linearity; the softmax
   denominator cancels in the K_bound l2-normalization so we use plain exp.
 - Host weight foldings remove two full projection stages: K is never
   materialized (Kf = kT.T @ (Wk F) + bk F and hidden = relu(kT.T @ (Wk Wr1)
   + bk Wr1 + br1)), and the output projection folds into V (VW = vT.T @
   (Wv Wo); the bv term becomes a constant output bias because the attention
   rows are normalized).
 - Per-token normalizations are folded: 1/||K_bound|| becomes a per-partition
   scalar in the scores epilogue (scores computed keys-major), 1/||Q|| scales
   q_n in SBUF, and the attention row-sum is folded into the output epilogue.

Everything is feature-major ("xT" = [feature, token]) so every chained matmul
is out = lhsT.T @ rhs with contraction on the partition dim.
"""

import sys

for _p in ("/opt/trn_rl_repo", "/root/.axon_site/_ro/trn_rl_repo"):
    if _p not in sys.path:
        sys.path.append(_p)

import numpy as np
import ml_dtypes

import concourse.bass as bass
import concourse.bacc as bacc
import concourse.tile as tile
from concourse import mybir
from concourse.bass_utils import run_bass_kernel_spmd

BF16 = ml_dtypes.bfloat16
F32 = mybir.dt.float32
BF = mybir.dt.bfloat16

B, TQ, TK, D, R, H = 4, 2048, 2048, 1024, 64, 512
TAU_BASE, ASTRO_DECAY = 1.0, 0.95
P = 128
NJ = D // P            # 8 feature chunks
NT = TK // 512         # 4 token chunks of 512
TQH = TQ // 2          # per-core query rows (1024)
NQC = TQH // 512       # 2 query chunks
NKT = TK // P          # 16 key tiles of 128
AF = mybir.ActivationFunctionType
ALU = mybir.AluOpType


# ---------------------------------------------------------------- host consts
def _build_dft_mats(n=1024):
    j = np.arange(n)[:, None].astype(np.float64)
    f = np.arange(n // 2)[None, :].astype(np.float64)
    F = np.zeros((n, n), np.float64)
    F[:, :512] = np.cos(2 * np.pi * j * f / n)
    F[:, 512] = np.cos(np.pi * j[:, 0])
    fi = np.arange(1, 512)[None, :].astype(np.float64)
    F[:, 513:] = -np.sin(2 * np.pi * j * fi / n)

    G = np.zeros((n, n), np.float64)
    d = np.arange(n)[None, :].astype(np.float64)
    G[0, :] = 1.0 / n
    ff = np.arange(1, 512)[:, None].astype(np.float64)
    G[1:512, :] = 2.0 * np.cos(2 * np.pi * ff * d / n) / n
    G[512, :] = ((-1.0) ** d[0]) / n
    G[513:, :] = -2.0 * np.sin(2 * np.pi * ff * d / n) / n
    return F.astype(np.float32), G.astype(np.float32)


def _pack_rfft(x):
    X = np.fft.rfft(np.asarray(x, np.float64), axis=-1)
    out = np.empty(x.shape, np.float32)
    out[..., :512] = X.real[..., :512]
    out[..., 512] = X.real[..., 512]
    out[..., 513:] = X.imag[..., 1:512]
    return out


# ---------------------------------------------------------------- bass kernel
def _emit(nc: bass.Bass):
    # ---- I/O declarations (names = in_map keys) ----
    din = {}
    for nm, shp, dt in [
        ("qT", [D, TQ], BF), ("kT", [D, TK], BF), ("vT", [D, TK], BF),
        ("wq", [D, D], BF), ("wkf", [D, D], BF), ("gmat", [D, D], BF),
        ("vwo", [D, D], BF), ("wkr1", [D, H], BF), ("wr2", [H, R], BF),
        ("pf", [R, D], BF),
        ("bq", [P, NJ], F32), ("bkf", [P, NJ], F32), ("bvo", [P, NJ], F32),
        ("brk", [P, H // P], F32), ("scal", [1, 2], F32),
    ]:
        din[nm] = nc.declare_dram_parameter(nm, shp, dt, isOutput=False)
    outT = nc.declare_dram_parameter("outT", [D, TQH], F32, isOutput=True)
    ns_out = nc.declare_dram_parameter("ns_out", [1, 1], F32, isOutput=True)

    # internal DRAM spill tensors
    kn_d = nc.dram_tensor("kn_d", [D, TK], BF)
    VW_d = nc.dram_tensor("VW_d", [TK, D], BF)
    rk_stage = nc.dram_tensor("rk_stage", [1, TK], F32)
    rq_stage = nc.dram_tensor("rq_stage", [1, TQH], F32)
    rs_stage = nc.dram_tensor("rs_stage", [1, 2 * 512], F32)
    c_stage = nc.dram_tensor("c_stage", [1, 1], F32)

    c3 = lambda t: t.rearrange("(po pi) x -> pi po x", pi=P)

    with tile.TileContext(nc) as tc, \
         tc.tile_pool(name="w", bufs=2) as wpool, \
         tc.tile_pool(name="wsmall", bufs=1) as wsmall, \
         tc.tile_pool(name="instream", bufs=2) as instream, \
         tc.tile_pool(name="outstage", bufs=3) as outstage, \
         tc.tile_pool(name="persist", bufs=1) as persist, \
         tc.tile_pool(name="kfc", bufs=2) as kfcpool, \
         tc.tile_pool(name="scores", bufs=2) as scorepool, \
         tc.tile_pool(name="attst", bufs=2) as attst, \
         tc.tile_pool(name="tmp", bufs=2) as tmp, \
         tc.tile_pool(name="cmt", bufs=2) as cmt, \
         tc.tile_pool(name="small", bufs=1) as small, \
         tc.tile_pool(name="rows", bufs=1) as rows, \
         tc.tile_pool(name="ps", bufs=4, space="PSUM") as ps, \
         tc.tile_pool(name="psrow", bufs=3, space="PSUM") as psrow:

        # ---- constants ----
        ones_k = small.tile([P, 1], BF, tag="ones_k")      # lhsT for partition sums
        nc.vector.memset(ones_k, 1.0)
        bq_sb = small.tile([P, NJ], F32, tag="bq")
        nc.sync.dma_start(bq_sb, din["bq"][:])
        bkf_sb = small.tile([P, NJ], F32, tag="bkf")
        nc.sync.dma_start(bkf_sb, din["bkf"][:])
        bvo_sb = small.tile([P, NJ], F32, tag="bvo")
        nc.sync.dma_start(bvo_sb, din["bvo"][:])
        brk_sb = small.tile([P, H // P], F32, tag="brk")
        nc.sync.dma_start(brk_sb, din["brk"][:])
        scal_sb = small.tile([1, 2], F32, tag="scal")
        nc.sync.dma_start(scal_sb, din["scal"][:])

        # pre-touch const tiles so later consumers don't accumulate one sync
        # wait per DMA queue on top of their compute waits (ACT has a small
        # hardware wait table)
        pre_s = small.tile([P, 1], F32, tag="pre_s")
        nc.scalar.activation(pre_s, bq_sb[:, 0:1], AF.Identity, bias=bkf_sb[:, 0:1])
        nc.scalar.activation(pre_s, pre_s, AF.Relu, bias=brk_sb[:, 0:1])
        pre_v = small.tile([P, 1], F32, tag="pre_v")
        nc.vector.tensor_scalar_add(pre_v, pre_s, bvo_sb[:, 0:1])
        pre_v2 = small.tile([1, 1], F32, tag="pre_v2")
        nc.vector.tensor_scalar_add(pre_v2, scal_sb[0:1, 0:1], 0.0)

        def load_w(name, shape3):
            t = wpool.tile(shape3, BF, tag="w", name=name)
            nc.sync.dma_start(t, c3(din[name][:]))
            return t

        # ==================== S_A: hidden = relu(WkR1.T @ kT + brk)
        wkr1 = wsmall.tile([P, NJ, H], BF, tag="wkr1")
        nc.sync.dma_start(wkr1, c3(din["wkr1"][:]))
        hidden = scorepool.tile([P, H // P, TK], BF, tag="scT", name="hidden")
        for t in range(NT):
            kin = instream.tile([P, NJ, 512], BF, tag="instream")
            nc.sync.dma_start(kin, c3(din["kT"][:])[:, :, t * 512:(t + 1) * 512])
            for jh in range(H // P):
                pst = ps.tile([P, 512], F32, tag="mm")
                for i in range(NJ):
                    nc.tensor.matmul(pst, wkr1[:, i, jh * P:(jh + 1) * P], kin[:, i, :],
                                     start=(i == 0), stop=(i == NJ - 1))
                nc.scalar.activation(hidden[:, jh, t * 512:(t + 1) * 512], pst,
                                     AF.Relu, bias=brk_sb[:, jh:jh + 1])

        # ==================== S_C: role logits -> exp -> Rf (SBUF resident)
        wr2 = wsmall.tile([P, H // P, R], BF, tag="wr2")
        nc.sync.dma_start(wr2, c3(din["wr2"][:]))
        rw = persist.tile([R, TK], BF, tag="rw")
        for t in range(NT):
            ps64 = ps.tile([R, 512], F32, tag="mm")
            for i2 in range(H // P):
                nc.tensor.matmul(ps64, wr2[:, i2, :], hidden[:, i2, t * 512:(t + 1) * 512],
                                 start=(i2 == 0), stop=(i2 == H // P - 1))
            nc.scalar.activation(rw[:, t * 512:(t + 1) * 512], ps64, AF.Exp)
        pf = wsmall.tile([R, D], BF, tag="pf")
        nc.sync.dma_start(pf, din["pf"][:])
        Rf = persist.tile([P, NJ, TK], BF, tag="big1", name="Rf")
        for t in range(NT):
            for j in range(NJ):
                pst = ps.tile([P, 512], F32, tag="mm")
                nc.tensor.matmul(pst, pf[:, j * P:(j + 1) * P],
                                 rw[:, t * 512:(t + 1) * 512], start=True, stop=True)
                nc.scalar.activation(Rf[:, j, t * 512:(t + 1) * 512], pst, AF.Copy)

        # ==================== S_B: fused Kf -> cmul -> KB per token chunk
        # Kf(t) matmuls, then (interleaved one chunk later so the DVE cmul
        # hides behind PE work) KB(t) = G.T @ Z(t) with Z computed in-place
        # in the kf chunk.  Packed pairing: tile j with tile j+4.
        wkf = load_w("wkf", [P, NJ, D])
        gmat = load_w("gmat", [P, NJ, D])

        kfc = [None] * NT

        def emit_kf(t):
            kin = instream.tile([P, NJ, 512], BF, tag="instream", name=f"kin2_{t}")
            nc.sync.dma_start(kin, c3(din["kT"][:])[:, :, t * 512:(t + 1) * 512])
            kfc[t] = kfcpool.tile([P, NJ, 512], BF, tag="kfc", name=f"kfc{t}")
            for j in range(NJ):
                pst = ps.tile([P, 512], F32, tag="mm")
                for i in range(NJ):
                    nc.tensor.matmul(pst, wkf[:, i, j * P:(j + 1) * P], kin[:, i, :],
                                     start=(i == 0), stop=(i == NJ - 1))
                nc.scalar.activation(kfc[t][:, j, :], pst, AF.Identity,
                                     bias=bkf_sb[:, j:j + 1])
            # cmul in place: kf chunk becomes Z chunk (DVE)
            kf = kfc[t]
            rfs = Rf[:, :, t * 512:(t + 1) * 512]
            zf0 = cmt.tile([1, 512], BF, tag="zf0")
            zf4 = cmt.tile([1, 512], BF, tag="zf4")
            nc.vector.tensor_mul(zf0, kf[0:1, 0, :], rfs[0:1, 0, :])
            nc.vector.tensor_mul(zf4, kf[0:1, 4, :], rfs[0:1, 4, :])
            for j in range(4):
                t2 = cmt.tile([P, 512], BF, tag="cm_t2")
                t3 = cmt.tile([P, 512], BF, tag="cm_t3")
                nc.vector.tensor_mul(t2, kf[:, j + 4, :], rfs[:, j + 4, :])
                nc.vector.tensor_mul(t3, kf[:, j + 4, :], rfs[:, j, :])
                nc.vector.tensor_mul(kf[:, j + 4, :], kf[:, j, :], rfs[:, j + 4, :])
                nc.vector.tensor_tensor(kf[:, j + 4, :], kf[:, j + 4, :], t3, ALU.add)
                nc.vector.tensor_mul(kf[:, j, :], kf[:, j, :], rfs[:, j, :])
                nc.vector.tensor_tensor(kf[:, j, :], kf[:, j, :], t2, ALU.subtract)
            # rows 0 (re_0) and 512 (re_512) are purely real
            nc.vector.tensor_copy(kf[0:1, 0, :], zf0)
            nc.vector.tensor_copy(kf[0:1, 4, :], zf4)

        def emit_kb(t):
            psn = psrow.tile([1, 512], F32, tag="rowsum", name=f"psn_kb{t}")
            for j in range(NJ):
                pst = ps.tile([P, 512], F32, tag="mm")
                for i in range(NJ):
                    nc.tensor.matmul(pst, gmat[:, i, j * P:(j + 1) * P], kfc[t][:, i, :],
                                     start=(i == 0), stop=(i == NJ - 1))
                kb_o = outstage.tile([P, 512], BF, tag="outstage")
                nc.scalar.activation(kb_o, pst, AF.Copy)
                nc.sync.dma_start(kn_d[j * P:(j + 1) * P, t * 512:(t + 1) * 512], kb_o)
                sq = tmp.tile([P, 512], BF, tag="sq")
                nc.vector.tensor_mul(sq, kb_o, kb_o)
                nc.tensor.matmul(psn, ones_k, sq, start=(j == 0), stop=(j == NJ - 1))
            # rk chunk = 1/sqrt(norm2) -> stage to DRAM for the column reload
            srow = rows.tile([1, 512], F32, tag="srow")
            nc.scalar.activation(srow, psn, AF.Sqrt)
            rrow = rows.tile([1, 512], F32, tag="rrow")
            nc.vector.reciprocal(rrow, srow)
            nc.sync.dma_start(rk_stage[0:1, t * 512:(t + 1) * 512], rrow)

        emit_kf(0)
        emit_kf(1)
        emit_kb(0)
        emit_kf(2)
        emit_kb(1)
        emit_kf(3)
        emit_kb(2)
        emit_kb(3)
        # rk as [128,16] per-key-tile partition scalars, negated for the
        # (S*(-rk) + 1) epilogue
        rk_col = small.tile([P, NKT], F32, tag="rk_col")
        nc.sync.dma_start(
            rk_col, rk_stage.rearrange("o (kt p) -> (o p) kt", p=P))
        nrk_col = small.tile([P, NKT], F32, tag="nrk_col")
        nc.vector.tensor_scalar_mul(nrk_col, rk_col, -1.0)

        # ==================== S_E: VW = vT.T @ (WvWo)
        vwo = load_w("vwo", [P, NJ, D])
        for to in range(NKT):
            vin = attst.tile([P, NJ, P], BF, tag="knstream", name="vin")
            nc.sync.dma_start(vin, c3(din["vT"][:])[:, :, to * P:(to + 1) * P])
            for n in range(2):
                pst = ps.tile([P, 512], F32, tag="mm")
                for i in range(NJ):
                    nc.tensor.matmul(pst, vin[:, i, :], vwo[:, i, n * 512:(n + 1) * 512],
                                     start=(i == 0), stop=(i == NJ - 1))
                v_o = outstage.tile([P, 512], BF, tag="outstage")
                nc.scalar.activation(v_o, pst, AF.Copy)
                nc.sync.dma_start(VW_d[to * P:(to + 1) * P, n * 512:(n + 1) * 512], v_o)

        # ==================== S_F: Q proj (full TQ), tau, q_n
        wq = load_w("wq", [P, NJ, D])
        qn = persist.tile([P, NJ, TQH], BF, tag="big1", name="qn")
        rq_bc = small.tile([P, TQH], F32, tag="rq_bc")
        ssc = small.tile([1, NT], F32, tag="ssc")
        for t in range(NT):  # over full TQ; chunks 0,1 are this core's half
            qin = instream.tile([P, NJ, 512], BF, tag="qstream", name="qin")
            nc.sync.dma_start(qin, c3(din["qT"][:])[:, :, t * 512:(t + 1) * 512])
            psn = psrow.tile([1, 512], F32, tag="rowsum")
            for j in range(NJ):
                pst = ps.tile([P, 512], F32, tag="mm")
                for i in range(NJ):
                    nc.tensor.matmul(pst, wq[:, i, j * P:(j + 1) * P], qin[:, i, :],
                                     start=(i == 0), stop=(i == NJ - 1))
                if t < NQC:
                    qv = qn[:, j, t * 512:(t + 1) * 512]
                else:
                    qv = tmp.tile([P, 512], BF, tag="qv_other")
                nc.scalar.activation(qv, pst, AF.Identity, bias=bq_sb[:, j:j + 1])
                sq = tmp.tile([P, 512], BF, tag="sq")
                nc.vector.tensor_mul(sq, qv, qv)
                nc.tensor.matmul(psn, ones_k, sq, start=(j == 0), stop=(j == NJ - 1))
            # sqrt of norms; accumulate sum(sqrt) for the surprise mean
            srow = rows.tile([1, 512], F32, tag="srow")
            nc.scalar.activation(srow, psn, AF.Sqrt, accum_out=ssc[0:1, t:t + 1])
            if t < NQC:  # own half: 1/norm, staged for the rq broadcast
                rrow = rows.tile([1, 512], F32, tag="rrow")
                nc.vector.reciprocal(rrow, srow)
                nc.sync.dma_start(rq_stage[0:1, t * 512:(t + 1) * 512], rrow)
        nc.sync.dma_start(
            rq_bc, bass.AP(tensor=rq_stage, offset=0, ap=[[0, P], [1, TQH]]))
        ss = small.tile([1, 1], F32, tag="ss")
        nc.vector.reduce_sum(ss, ssc, axis=mybir.AxisListType.X)
        # new_state = 0.95*astro + 0.05*ss/(32*2048)
        ns_t = small.tile([1, 1], F32, tag="ns_t")
        v1 = small.tile([1, 1], F32, tag="v1")
        nc.vector.tensor_scalar_mul(v1, ss, (1.0 - ASTRO_DECAY) / (32.0 * TQ))
        v2 = small.tile([1, 1], F32, tag="v2")
        nc.vector.tensor_scalar_mul(v2, scal_sb[0:1, 0:1], ASTRO_DECAY)
        nc.vector.tensor_add(ns_t, v1, v2)
        nc.sync.dma_start(ns_out[:], ns_t)
        # c = tau/4 = max(1 + astro_scale*ns, 0.001)/4
        c_t = small.tile([1, 1], F32, tag="c_t")
        nc.vector.tensor_mul(c_t, ns_t, scal_sb[0:1, 1:2])
        nc.vector.tensor_scalar(c_t, c_t, 1.0, 0.001, ALU.add, ALU.max)
        nc.vector.tensor_scalar_mul(c_t, c_t, 0.25 * TAU_BASE)
        nc.sync.dma_start(c_stage[:], c_t)
        c_sb = small.tile([P, 1], F32, tag="c_sb")
        nc.sync.dma_start(
            c_sb, bass.AP(tensor=c_stage, offset=0, ap=[[0, P], [1, 1]]))
        for j in range(NJ):
            nc.vector.tensor_mul(qn[:, j, :], qn[:, j, :], rq_bc)

        # ==================== S_H: attention scores (kt outer, both qc)
        scT = [scorepool.tile([P, NKT, 512], BF, tag="scT", name=f"scT{qc}")
               for qc in range(NQC)]
        rs_ps = [psrow.tile([1, 512], F32, tag="rowsum", name=f"rs_ps{qc}")
                 for qc in range(NQC)]
        for kt in range(NKT):
            knc = attst.tile([P, NJ, P], BF, tag="knstream", name="knc")
            nc.sync.dma_start(knc, c3(kn_d[:])[:, :, kt * P:(kt + 1) * P])
            for qc in range(NQC):
                ps_s = ps.tile([P, 512], F32, tag="mm")
                for j in range(NJ):
                    nc.tensor.matmul(ps_s, knc[:, j, :], qn[:, j, qc * 512:(qc + 1) * 512],
                                     start=(j == 0), stop=(j == NJ - 1))
                u0 = tmp.tile([P, 512], BF, tag="u0")
                # u0 = 1 - cos = S*(-rk[key]) + 1
                nc.vector.tensor_scalar(u0, ps_s, nrk_col[:, kt:kt + 1], 1.0,
                                        ALU.mult, ALU.add)
                u1 = tmp.tile([P, 512], BF, tag="u1")
                nc.scalar.activation(u1, u0, AF.Square)
                u2 = tmp.tile([P, 512], BF, tag="u2")
                nc.vector.tensor_scalar_mul(u2, u1, c_sb)
                # scores = relu(1 - c*(1-cos)^2)
                nc.scalar.activation(scT[qc][:, kt, :], u2, AF.Relu,
                                     bias=1.0, scale=-1.0)
                nc.tensor.matmul(rs_ps[qc], ones_k, scT[qc][:, kt, :],
                                 start=(kt == 0), stop=(kt == NKT - 1))
        # rowsum reciprocal -> broadcast [128,512] per qc
        rs_bc = []
        for qc in range(NQC):
            rs_row = small.tile([1, 512], F32, tag=f"rs_row{qc}", name=f"rs_row{qc}")
            nc.vector.reciprocal(rs_row, rs_ps[qc])
            nc.sync.dma_start(rs_stage[0:1, qc * 512:(qc + 1) * 512], rs_row)
            rb = small.tile([P, 512], F32, tag=f"rs_bc{qc}", name=f"rs_bc{qc}")
            nc.sync.dma_start(
                rb, bass.AP(tensor=rs_stage, offset=qc * 512, ap=[[0, P], [1, 512]]))
            rs_bc.append(rb)

        # ==================== S_I: out = (scT.T @ VW)*rs + bvo  (Wo folded in)
        for qc in range(NQC):
            for j2g in range(2):  # Dout groups of 512 (4 psum tiles each)
                pcs = [ps.tile([P, 512], F32, tag="mm", name=f"pcs{i}") for i in range(4)]
                for kt in range(NKT):
                    vkt = attst.tile([P, 512], BF, tag="vstream8", name="vkt")
                    nc.sync.dma_start(
                        vkt, VW_d[kt * P:(kt + 1) * P, j2g * 512:(j2g + 1) * 512])
                    for jj in range(4):
                        nc.tensor.matmul(pcs[jj], vkt[:, jj * P:(jj + 1) * P],
                                         scT[qc][:, kt, :],
                                         start=(kt == 0), stop=(kt == NKT - 1))
                for jj in range(4):
                    j = j2g * 4 + jj
                    o_t = tmp.tile([P, 512], F32, tag="o_t")
                    nc.vector.tensor_mul(o_t, pcs[jj], rs_bc[qc])
                    nc.vector.tensor_scalar_add(o_t, o_t, bvo_sb[:, j:j + 1])
                    nc.sync.dma_start(
                        outT[j * P:(j + 1) * P, qc * 512:(qc + 1) * 512], o_t)

    return nc


_CACHE = {}


def _get_nc():
    if "nc" not in _CACHE:
        nc = bacc.Bacc(None, target_bir_lowering=False)
        _emit(nc)
        nc.finalize()
        _CACHE["nc"] = nc
    return _CACHE["nc"]


def build_in_maps(inputs):
    """Host-side prep: foldings, packing, per-core sharding."""
    F, G = _build_dft_mats(D)
    role = np.asarray(inputs["role_matrix"], np.float32)
    role = role / np.clip(np.linalg.norm(role, axis=-1, keepdims=True), 1e-12, None)
    PF = _pack_rfft(role)

    f32 = lambda x: np.asarray(x, np.float32)
    bf = lambda x: np.ascontiguousarray(f32(x)).astype(BF16)
    btile = lambda x: np.ascontiguousarray(f32(x).reshape(-1, P).T.copy())

    Wk, Wv, Wo, Wr1 = f32(inputs["Wk"]), f32(inputs["Wv"]), f32(inputs["Wo"]), f32(inputs["Wr1"])
    bk, bv, bo, br1 = f32(inputs["bk"]), f32(inputs["bv"]), f32(inputs["bo"]), f32(inputs["br1"])

    weights = {
        "wq": bf(inputs["Wq"]),
        "wkf": bf(Wk @ F),
        "gmat": bf(G),
        "vwo": bf(Wv @ Wo),
        "wkr1": bf(Wk @ Wr1),
        "wr2": bf(inputs["Wr2"]),
        "pf": bf(PF),
        "bq": btile(inputs["bq"]),
        "bkf": btile(bk @ F),
        "bvo": btile(bv @ Wo + bo),
        "brk": btile(bk @ Wr1 + br1),
    }

    in_maps = []
    for core in range(8):
        b, h = core // 2, core % 2
        q = f32(inputs["q_in"][b])
        own = q[h * TQH:(h + 1) * TQH]
        other = q[(1 - h) * TQH:(2 - h) * TQH]
        m = dict(weights)
        m["qT"] = np.ascontiguousarray(np.concatenate([own, other], 0).T).astype(BF16)
        m["kT"] = np.ascontiguousarray(f32(inputs["k_in"][b]).T).astype(BF16)
        m["vT"] = np.ascontiguousarray(f32(inputs["v_in"][b]).T).astype(BF16)
        m["scal"] = np.array(
            [[np.float32(inputs["astrocyte_state"][b]),
              np.float32(np.asarray(inputs["astro_scale"]).reshape(-1)[0])]],
            np.float32)
        in_maps.append(m)
    return in_maps


def kernel(q_in, k_in, v_in, astrocyte_state, Wq, bq, Wk, bk, Wv, bv, Wo, bo,
           role_matrix, Wr1, br1, Wr2, astro_scale, **_ignored):
    nc = _get_nc()
    inputs = dict(q_in=q_in, k_in=k_in, v_in=v_in, astrocyte_state=astrocyte_state,
                  Wq=Wq, bq=bq, Wk=Wk, bk=bk, Wv=Wv, bv=bv, Wo=Wo, bo=bo,
                  role_matrix=role_matrix, Wr1=Wr1, br1=br1, Wr2=Wr2,
                  astro_scale=astro_scale)
    in_maps = build_in_maps(inputs)
    res = run_bass_kernel_spmd(nc, in_maps, core_ids=list(range(8)))

    output = np.empty((B, TQ, D), np.float32)
    new_state = np.empty((B,), np.float32)
    for core in range(8):
        b, h = core // 2, core % 2
        output[b, h * TQH:(h + 1) * TQH, :] = res.results[core]["outT"].T
        if h == 0:
            new_state[b] = res.results[core]["ns_out"][0, 0]
    return output, new_state


# revision 25
# speedup vs baseline: 1.4640x; 1.0532x over previous
"""Trainium2 Bass kernel for nn_AstroSymbolicEpisodicLayer.

Strategy
--------
8 cores = (batch b in 0..3) x (query-half h in 0..1). One SPMD program; all
per-core differences are data (each core's qT has its own query half first).

Math decomposition (validated to ~1e-6 in fp32 vs the jax reference):
 - Circular-convolution binding via a packed real DFT implemented as
   1024x1024 matmuls.  Packed layout: rows 0..511 are Re bins 0..511, row 512
   is Re bin 512, rows 513..1023 are Im bins 1..511.  Pointwise complex
   multiply pairs row f with row 512+f (tile j with tile j+4), plus a 2-row
   fixup for the purely-real bins 0 and 512.
 - rfft(role_vecs) = role_weights @ rfft(role_norm) by linearity; the softmax
   denominator cancels in the K_bound l2-normalization so we use plain exp.
 - Host weight foldings remove two full projection stages: K is never
   materialized (Kf = kT.T @ (Wk F) + bk F and hidden = relu(kT.T @ (Wk Wr1)
   + bk Wr1 + br1)), and the output projection folds into V (VW = vT.T @
   (Wv Wo); the bv term becomes a constant output bias because the attention
   rows are normalized).
 - Per-token normalizations are folded: 1/||K_bound|| becomes a per-partition
   scalar in the scores epilogue (scores computed keys-major), 1/||Q|| scales
   q_n in SBUF, and the attention row-sum is folded into the output epilogue.

Everything is feature-major ("xT" = [feature, token]) so every chained matmul
is out = lhsT.T @ rhs with contraction on the partition dim.
"""

import sys

for _p in ("/opt/trn_rl_repo", "/root/.axon_site/_ro/trn_rl_repo"):
    if _p not in sys.path:
        sys.path.append(_p)

import numpy as np
import ml_dtypes

import concourse.bass as bass
import concourse.bacc as bacc
import concourse.tile as tile
from concourse import mybir
from concourse.bass_utils import run_bass_kernel_spmd

BF16 = ml_dtypes.bfloat16
F32 = mybir.dt.float32
BF = mybir.dt.bfloat16

B, TQ, TK, D, R, H = 4, 2048, 2048, 1024, 64, 512
TAU_BASE, ASTRO_DECAY = 1.0, 0.95
P = 128
NJ = D // P            # 8 feature chunks
NT = TK // 512         # 4 token chunks of 512
TQH = TQ // 2          # per-core query rows (1024)
NQC = TQH // 512       # 2 query chunks
NKT = TK // P          # 16 key tiles of 128
AF = mybir.ActivationFunctionType
ALU = mybir.AluOpType


# ---------------------------------------------------------------- host consts
def _build_dft_mats(n=1024):
    j = np.arange(n)[:, None].astype(np.float64)
    f = np.arange(n // 2)[None, :].astype(np.float64)
    F = np.zeros((n, n), np.float64)
    F[:, :512] = np.cos(2 * np.pi * j * f / n)
    F[:, 512] = np.cos(np.pi * j[:, 0])
    fi = np.arange(1, 512)[None, :].astype(np.float64)
    F[:, 513:] = -np.sin(2 * np.pi * j * fi / n)

    G = np.zeros((n, n), np.float64)
    d = np.arange(n)[None, :].astype(np.float64)
    G[0, :] = 1.0 / n
    ff = np.arange(1, 512)[:, None].astype(np.float64)
    G[1:512, :] = 2.0 * np.cos(2 * np.pi * ff * d / n) / n
    G[512, :] = ((-1.0) ** d[0]) / n
    G[513:, :] = -2.0 * np.sin(2 * np.pi * ff * d / n) / n
    return F.astype(np.float32), G.astype(np.float32)


def _pack_rfft(x):
    X = np.fft.rfft(np.asarray(x, np.float64), axis=-1)
    out = np.empty(x.shape, np.float32)
    out[..., :512] = X.real[..., :512]
    out[..., 512] = X.real[..., 512]
    out[..., 513:] = X.imag[..., 1:512]
    return out


# ---------------------------------------------------------------- bass kernel
def _emit(nc: bass.Bass):
    # ---- I/O declarations (names = in_map keys) ----
    din = {}
    for nm, shp, dt in [
        ("qT", [D, TQ], BF), ("kT", [D, TK], BF), ("vT", [D, TK], BF),
        ("wq", [D, D], BF), ("wkf", [D, D], BF), ("gmat", [D, D], BF),
        ("vwo", [D, D], BF), ("wkr1", [D, H], BF), ("wr2", [H, R], BF),
        ("pf", [R, D], BF),
        ("bq", [P, NJ], F32), ("bkf", [P, NJ], F32), ("bvo", [P, NJ], F32),
        ("brk", [P, H // P], F32), ("scal", [1, 2], F32),
    ]:
        din[nm] = nc.declare_dram_parameter(nm, shp, dt, isOutput=False)
    outT = nc.declare_dram_parameter("outT", [D, TQH], F32, isOutput=True)
    ns_out = nc.declare_dram_parameter("ns_out", [1, 1], F32, isOutput=True)

    # internal DRAM spill tensors
    kn_d = nc.dram_tensor("kn_d", [D, TK], BF)
    VW_d = nc.dram_tensor("VW_d", [TK, D], BF)
    rk_stage = nc.dram_tensor("rk_stage", [1, TK], F32)
    rq_stage = nc.dram_tensor("rq_stage", [1, TQH], BF)
    rs_stage = nc.dram_tensor("rs_stage", [1, 2 * 512], F32)
    c_stage = nc.dram_tensor("c_stage", [1, 1], F32)

    c3 = lambda t: t.rearrange("(po pi) x -> pi po x", pi=P)

    with tile.TileContext(nc) as tc, \
         tc.tile_pool(name="w", bufs=2) as wpool, \
         tc.tile_pool(name="wsmall", bufs=1) as wsmall, \
         tc.tile_pool(name="instream", bufs=2) as instream, \
         tc.tile_pool(name="outstage", bufs=2) as outstage, \
         tc.tile_pool(name="persist", bufs=1) as persist, \
         tc.tile_pool(name="kfc", bufs=2) as kfcpool, \
         tc.tile_pool(name="scores", bufs=2) as scorepool, \
         tc.tile_pool(name="attst", bufs=3) as attst, \
         tc.tile_pool(name="tmp", bufs=2) as tmp, \
         tc.tile_pool(name="cmt", bufs=2) as cmt, \
         tc.tile_pool(name="small", bufs=1) as small, \
         tc.tile_pool(name="rows", bufs=1) as rows, \
         tc.tile_pool(name="ps", bufs=4, space="PSUM") as ps, \
         tc.tile_pool(name="psrow", bufs=3, space="PSUM") as psrow:

        # ---- constants ----
        ones_k = small.tile([P, 1], BF, tag="ones_k")      # lhsT for partition sums
        nc.vector.memset(ones_k, 1.0)
        bq_sb = small.tile([P, NJ], F32, tag="bq")
        nc.sync.dma_start(bq_sb, din["bq"][:])
        bkf_sb = small.tile([P, NJ], F32, tag="bkf")
        nc.sync.dma_start(bkf_sb, din["bkf"][:])
        bvo_sb = small.tile([P, NJ], F32, tag="bvo")
        nc.sync.dma_start(bvo_sb, din["bvo"][:])
        brk_sb = small.tile([P, H // P], F32, tag="brk")
        nc.sync.dma_start(brk_sb, din["brk"][:])
        scal_sb = small.tile([1, 2], F32, tag="scal")
        nc.sync.dma_start(scal_sb, din["scal"][:])

        # pre-touch const tiles so later consumers don't accumulate one sync
        # wait per DMA queue on top of their compute waits (ACT has a small
        # hardware wait table)
        pre_s = small.tile([P, 1], F32, tag="pre_s")
        nc.scalar.activation(pre_s, bq_sb[:, 0:1], AF.Identity, bias=bkf_sb[:, 0:1])
        nc.scalar.activation(pre_s, pre_s, AF.Relu, bias=brk_sb[:, 0:1])
        pre_v = small.tile([P, 1], F32, tag="pre_v")
        nc.vector.tensor_scalar_add(pre_v, pre_s, bvo_sb[:, 0:1])
        pre_v2 = small.tile([1, 1], F32, tag="pre_v2")
        nc.vector.tensor_scalar_add(pre_v2, scal_sb[0:1, 0:1], 0.0)

        def load_w(name, shape3):
            t = wpool.tile(shape3, BF, tag="w", name=name)
            nc.gpsimd.dma_start(t, c3(din[name][:]))
            return t

        # ==================== S_A: hidden = relu(WkR1.T @ kT + brk)
        wkr1 = wsmall.tile([P, NJ, H], BF, tag="wkr1")
        nc.gpsimd.dma_start(wkr1, c3(din["wkr1"][:]))
        hidden = scorepool.tile([P, H // P, TK], BF, tag="scT", name="hidden")
        for t in range(NT):
            kin = instream.tile([P, NJ, 512], BF, tag="instream")
            nc.sync.dma_start(kin, c3(din["kT"][:])[:, :, t * 512:(t + 1) * 512])
            for jh in range(H // P):
                pst = ps.tile([P, 512], F32, tag="mm")
                for i in range(NJ):
                    nc.tensor.matmul(pst, wkr1[:, i, jh * P:(jh + 1) * P], kin[:, i, :],
                                     start=(i == 0), stop=(i == NJ - 1))
                nc.scalar.activation(hidden[:, jh, t * 512:(t + 1) * 512], pst,
                                     AF.Relu, bias=brk_sb[:, jh:jh + 1])

        # ==================== S_C: role logits -> exp -> Rf (SBUF resident)
        wr2 = wsmall.tile([P, H // P, R], BF, tag="wr2")
        nc.gpsimd.dma_start(wr2, c3(din["wr2"][:]))
        rw = persist.tile([R, TK], BF, tag="rw")
        for t in range(NT):
            ps64 = ps.tile([R, 512], F32, tag="mm")
            for i2 in range(H // P):
                nc.tensor.matmul(ps64, wr2[:, i2, :], hidden[:, i2, t * 512:(t + 1) * 512],
                                 start=(i2 == 0), stop=(i2 == H // P - 1))
            nc.scalar.activation(rw[:, t * 512:(t + 1) * 512], ps64, AF.Exp)
        pf = wsmall.tile([R, D], BF, tag="pf")
        nc.gpsimd.dma_start(pf, din["pf"][:])
        Rf = persist.tile([P, NJ, TK], BF, tag="big1", name="Rf")
        for t in range(NT):
            for j in range(NJ):
                pst = ps.tile([P, 512], F32, tag="mm")
                nc.tensor.matmul(pst, pf[:, j * P:(j + 1) * P],
                                 rw[:, t * 512:(t + 1) * 512], start=True, stop=True)
                nc.scalar.activation(Rf[:, j, t * 512:(t + 1) * 512], pst, AF.Copy)

        # ==================== S_B: fused Kf -> cmul -> KB per token chunk
        # Kf(t) matmuls, then (interleaved one chunk later so the DVE cmul
        # hides behind PE work) KB(t) = G.T @ Z(t) with Z computed in-place
        # in the kf chunk.  Packed pairing: tile j with tile j+4.
        wkf = load_w("wkf", [P, NJ, D])
        gmat = load_w("gmat", [P, NJ, D])

        kfc = [None] * NT

        def emit_kf(t):
            kin = instream.tile([P, NJ, 512], BF, tag="instream", name=f"kin2_{t}")
            nc.sync.dma_start(kin, c3(din["kT"][:])[:, :, t * 512:(t + 1) * 512])
            kfc[t] = kfcpool.tile([P, NJ, 512], BF, tag="kfc", name=f"kfc{t}")
            for j in range(NJ):
                pst = ps.tile([P, 512], F32, tag="mm")
                for i in range(NJ):
                    nc.tensor.matmul(pst, wkf[:, i, j * P:(j + 1) * P], kin[:, i, :],
                                     start=(i == 0), stop=(i == NJ - 1))
                nc.scalar.activation(kfc[t][:, j, :], pst, AF.Identity,
                                     bias=bkf_sb[:, j:j + 1])
            # cmul in place: kf chunk becomes Z chunk (DVE)
            kf = kfc[t]
            rfs = Rf[:, :, t * 512:(t + 1) * 512]
            zf0 = cmt.tile([1, 512], BF, tag="zf0")
            zf4 = cmt.tile([1, 512], BF, tag="zf4")
            nc.vector.tensor_mul(zf0, kf[0:1, 0, :], rfs[0:1, 0, :])
            nc.vector.tensor_mul(zf4, kf[0:1, 4, :], rfs[0:1, 4, :])
            for j in range(4):
                t2 = cmt.tile([P, 512], BF, tag="cm_t2")
                t3 = cmt.tile([P, 512], BF, tag="cm_t3")
                nc.vector.tensor_mul(t2, kf[:, j + 4, :], rfs[:, j + 4, :])
                nc.vector.tensor_mul(t3, kf[:, j + 4, :], rfs[:, j, :])
                nc.vector.tensor_mul(kf[:, j + 4, :], kf[:, j, :], rfs[:, j + 4, :])
                nc.vector.tensor_tensor(kf[:, j + 4, :], kf[:, j + 4, :], t3, ALU.add)
                nc.vector.tensor_mul(kf[:, j, :], kf[:, j, :], rfs[:, j, :])
                nc.vector.tensor_tensor(kf[:, j, :], kf[:, j, :], t2, ALU.subtract)
            # rows 0 (re_0) and 512 (re_512) are purely real
            nc.vector.tensor_copy(kf[0:1, 0, :], zf0)
            nc.vector.tensor_copy(kf[0:1, 4, :], zf4)

        def emit_kb(t):
            psn = psrow.tile([1, 512], F32, tag="rowsum", name=f"psn_kb{t}")
            for j in range(NJ):
                pst = ps.tile([P, 512], F32, tag="mm")
                for i in range(NJ):
                    nc.tensor.matmul(pst, gmat[:, i, j * P:(j + 1) * P], kfc[t][:, i, :],
                                     start=(i == 0), stop=(i == NJ - 1))
                kb_o = outstage.tile([P, 512], BF, tag="outstage")
                nc.scalar.activation(kb_o, pst, AF.Copy)
                nc.gpsimd.dma_start(kn_d[j * P:(j + 1) * P, t * 512:(t + 1) * 512], kb_o)
                sq = tmp.tile([P, 512], BF, tag="sq")
                nc.vector.tensor_mul(sq, kb_o, kb_o)
                nc.tensor.matmul(psn, ones_k, sq, start=(j == 0), stop=(j == NJ - 1))
            # rk chunk = 1/sqrt(norm2) -> stage to DRAM for the column reload
            srow = rows.tile([1, 512], F32, tag="srow")
            nc.scalar.activation(srow, psn, AF.Sqrt)
            rrow = rows.tile([1, 512], F32, tag="rrow")
            nc.vector.reciprocal(rrow, srow)
            nc.gpsimd.dma_start(rk_stage[0:1, t * 512:(t + 1) * 512], rrow)

        emit_kf(0)
        emit_kf(1)
        emit_kb(0)
        emit_kf(2)
        emit_kb(1)
        emit_kf(3)
        emit_kb(2)
        emit_kb(3)
        # rk as [128,16] per-key-tile partition scalars, negated for the
        # (S*(-rk) + 1) epilogue
        rk_col = small.tile([P, NKT], F32, tag="rk_col")
        nc.sync.dma_start(
            rk_col, rk_stage.rearrange("o (kt p) -> (o p) kt", p=P))
        nrk_col = small.tile([P, NKT], F32, tag="nrk_col")
        nc.vector.tensor_scalar_mul(nrk_col, rk_col, -1.0)

        # ==================== S_E: VW = vT.T @ (WvWo)
        vwo = load_w("vwo", [P, NJ, D])
        for to in range(NKT):
            vin = attst.tile([P, NJ, P], BF, tag="knstream", name="vin")
            nc.sync.dma_start(vin, c3(din["vT"][:])[:, :, to * P:(to + 1) * P])
            for n in range(2):
                pst = ps.tile([P, 512], F32, tag="mm")
                for i in range(NJ):
                    nc.tensor.matmul(pst, vin[:, i, :], vwo[:, i, n * 512:(n + 1) * 512],
                                     start=(i == 0), stop=(i == NJ - 1))
                v_o = outstage.tile([P, 512], BF, tag="outstage")
                nc.scalar.activation(v_o, pst, AF.Copy)
                nc.gpsimd.dma_start(VW_d[to * P:(to + 1) * P, n * 512:(n + 1) * 512], v_o)

        # ==================== S_F: Q proj (full TQ), tau, q_n
        wq = load_w("wq", [P, NJ, D])
        qn = persist.tile([P, NJ, TQH], BF, tag="big1", name="qn")
        rq_bc = small.tile([P, TQH], BF, tag="rq_bc")
        ssc = small.tile([1, NT], F32, tag="ssc")
        for t in range(NT):  # over full TQ; chunks 0,1 are this core's half
            qin = instream.tile([P, NJ, 512], BF, tag="qstream", name="qin")
            nc.sync.dma_start(qin, c3(din["qT"][:])[:, :, t * 512:(t + 1) * 512])
            psn = psrow.tile([1, 512], F32, tag="rowsum")
            for j in range(NJ):
                pst = ps.tile([P, 512], F32, tag="mm")
                for i in range(NJ):
                    nc.tensor.matmul(pst, wq[:, i, j * P:(j + 1) * P], qin[:, i, :],
                                     start=(i == 0), stop=(i == NJ - 1))
                if t < NQC:
                    qv = qn[:, j, t * 512:(t + 1) * 512]
                else:
                    qv = tmp.tile([P, 512], BF, tag="qv_other")
                nc.scalar.activation(qv, pst, AF.Identity, bias=bq_sb[:, j:j + 1])
                sq = tmp.tile([P, 512], BF, tag="sq")
                nc.vector.tensor_mul(sq, qv, qv)
                nc.tensor.matmul(psn, ones_k, sq, start=(j == 0), stop=(j == NJ - 1))
            # sqrt of norms; accumulate sum(sqrt) for the surprise mean
            srow = rows.tile([1, 512], F32, tag="srow")
            nc.scalar.activation(srow, psn, AF.Sqrt, accum_out=ssc[0:1, t:t + 1])
            if t < NQC:  # own half: 1/norm, staged for the rq broadcast
                rrow = rows.tile([1, 512], F32, tag="rrow")
                nc.vector.reciprocal(rrow, srow)
                rrow16 = rows.tile([1, 512], BF, tag="rrow16")
                nc.vector.tensor_copy(rrow16, rrow)
                nc.gpsimd.dma_start(rq_stage[0:1, t * 512:(t + 1) * 512], rrow16)
        nc.sync.dma_start(
            rq_bc, bass.AP(tensor=rq_stage, offset=0, ap=[[0, P], [1, TQH]]))
        ss = small.tile([1, 1], F32, tag="ss")
        nc.vector.reduce_sum(ss, ssc, axis=mybir.AxisListType.X)
        # new_state = 0.95*astro + 0.05*ss/(32*2048)
        ns_t = small.tile([1, 1], F32, tag="ns_t")
        v1 = small.tile([1, 1], F32, tag="v1")
        nc.vector.tensor_scalar_mul(v1, ss, (1.0 - ASTRO_DECAY) / (32.0 * TQ))
        v2 = small.tile([1, 1], F32, tag="v2")
        nc.vector.tensor_scalar_mul(v2, scal_sb[0:1, 0:1], ASTRO_DECAY)
        nc.vector.tensor_add(ns_t, v1, v2)
        nc.gpsimd.dma_start(ns_out[:], ns_t)
        # c = tau/4 = max(1 + astro_scale*ns, 0.001)/4
        c_t = small.tile([1, 1], F32, tag="c_t")
        nc.vector.tensor_mul(c_t, ns_t, scal_sb[0:1, 1:2])
        nc.vector.tensor_scalar(c_t, c_t, 1.0, 0.001, ALU.add, ALU.max)
        nc.vector.tensor_scalar_mul(c_t, c_t, 0.25 * TAU_BASE)
        nc.gpsimd.dma_start(c_stage[:], c_t)
        c_sb = small.tile([P, 1], F32, tag="c_sb")
        nc.sync.dma_start(
            c_sb, bass.AP(tensor=c_stage, offset=0, ap=[[0, P], [1, 1]]))
        for j in range(NJ):
            nc.vector.tensor_mul(qn[:, j, :], qn[:, j, :], rq_bc)

        # ==================== S_H: attention scores (kt outer, both qc)
        scT = [scorepool.tile([P, NKT, 512], BF, tag="scT", name=f"scT{qc}")
               for qc in range(NQC)]
        rs_ps = [psrow.tile([1, 512], F32, tag="rowsum", name=f"rs_ps{qc}")
                 for qc in range(NQC)]
        for kt in range(NKT):
            knc = attst.tile([P, NJ, P], BF, tag="knstream", name="knc")
            nc.sync.dma_start(knc, c3(kn_d[:])[:, :, kt * P:(kt + 1) * P])
            for qc in range(NQC):
                ps_s = ps.tile([P, 512], F32, tag="mm")
                for j in range(NJ):
                    nc.tensor.matmul(ps_s, knc[:, j, :], qn[:, j, qc * 512:(qc + 1) * 512],
                                     start=(j == 0), stop=(j == NJ - 1))
                u0 = tmp.tile([P, 512], BF, tag="u0")
                # u0 = 1 - cos = S*(-rk[key]) + 1
                nc.vector.tensor_scalar(u0, ps_s, nrk_col[:, kt:kt + 1], 1.0,
                                        ALU.mult, ALU.add)
                u1 = tmp.tile([P, 512], BF, tag="u1")
                nc.scalar.activation(u1, u0, AF.Square)
                u2 = tmp.tile([P, 512], BF, tag="u2")
                nc.vector.tensor_scalar_mul(u2, u1, c_sb)
                # scores = relu(1 - c*(1-cos)^2)
                nc.scalar.activation(scT[qc][:, kt, :], u2, AF.Relu,
                                     bias=1.0, scale=-1.0)
                nc.tensor.matmul(rs_ps[qc], ones_k, scT[qc][:, kt, :],
                                 start=(kt == 0), stop=(kt == NKT - 1))
        # rowsum reciprocal -> broadcast [128,512] per qc
        rs_bc = []
        for qc in range(NQC):
            rs_row = small.tile([1, 512], F32, tag=f"rs_row{qc}", name=f"rs_row{qc}")
            nc.vector.reciprocal(rs_row, rs_ps[qc])
            nc.gpsimd.dma_start(rs_stage[0:1, qc * 512:(qc + 1) * 512], rs_row)
            rb = small.tile([P, 512], F32, tag=f"rs_bc{qc}", name=f"rs_bc{qc}")
            nc.sync.dma_start(
                rb, bass.AP(tensor=rs_stage, offset=qc * 512, ap=[[0, P], [1, 512]]))
            rs_bc.append(rb)

        # ==================== S_I: out = (scT.T @ VW)*rs + bvo  (Wo folded in)
        for qc in range(NQC):
            for j2g in range(2):  # Dout groups of 512 (4 psum tiles each)
                pcs = [ps.tile([P, 512], F32, tag="mm", name=f"pcs{i}") for i in range(4)]
                for kt in range(NKT):
                    vkt = attst.tile([P, 512], BF, tag="vstream8", name="vkt")
                    nc.sync.dma_start(
                        vkt, VW_d[kt * P:(kt + 1) * P, j2g * 512:(j2g + 1) * 512])
                    for jj in range(4):
                        nc.tensor.matmul(pcs[jj], vkt[:, jj * P:(jj + 1) * P],
                                         scT[qc][:, kt, :],
                                         start=(kt == 0), stop=(kt == NKT - 1))
                for jj in range(4):
                    j = j2g * 4 + jj
                    o_t = tmp.tile([P, 512], F32, tag="o_t")
                    nc.vector.tensor_mul(o_t, pcs[jj], rs_bc[qc])
                    nc.vector.tensor_scalar_add(o_t, o_t, bvo_sb[:, j:j + 1])
                    nc.gpsimd.dma_start(
                        outT[j * P:(j + 1) * P, qc * 512:(qc + 1) * 512], o_t)

    return nc


_CACHE = {}


def _get_nc():
    if "nc" not in _CACHE:
        nc = bacc.Bacc(None, target_bir_lowering=False)
        _emit(nc)
        nc.finalize()
        _CACHE["nc"] = nc
    return _CACHE["nc"]


def build_in_maps(inputs):
    """Host-side prep: foldings, packing, per-core sharding."""
    F, G = _build_dft_mats(D)
    role = np.asarray(inputs["role_matrix"], np.float32)
    role = role / np.clip(np.linalg.norm(role, axis=-1, keepdims=True), 1e-12, None)
    PF = _pack_rfft(role)

    f32 = lambda x: np.asarray(x, np.float32)
    bf = lambda x: np.ascontiguousarray(f32(x)).astype(BF16)
    btile = lambda x: np.ascontiguousarray(f32(x).reshape(-1, P).T.copy())

    Wk, Wv, Wo, Wr1 = f32(inputs["Wk"]), f32(inputs["Wv"]), f32(inputs["Wo"]), f32(inputs["Wr1"])
    bk, bv, bo, br1 = f32(inputs["bk"]), f32(inputs["bv"]), f32(inputs["bo"]), f32(inputs["br1"])

    weights = {
        "wq": bf(inputs["Wq"]),
        "wkf": bf(Wk @ F),
        "gmat": bf(G),
        "vwo": bf(Wv @ Wo),
        "wkr1": bf(Wk @ Wr1),
        "wr2": bf(inputs["Wr2"]),
        "pf": bf(PF),
        "bq": btile(inputs["bq"]),
        "bkf": btile(bk @ F),
        "bvo": btile(bv @ Wo + bo),
        "brk": btile(bk @ Wr1 + br1),
    }

    in_maps = []
    for core in range(8):
        b, h = core // 2, core % 2
        q = f32(inputs["q_in"][b])
        own = q[h * TQH:(h + 1) * TQH]
        other = q[(1 - h) * TQH:(2 - h) * TQH]
        m = dict(weights)
        m["qT"] = np.ascontiguousarray(np.concatenate([own, other], 0).T).astype(BF16)
        m["kT"] = np.ascontiguousarray(f32(inputs["k_in"][b]).T).astype(BF16)
        m["vT"] = np.ascontiguousarray(f32(inputs["v_in"][b]).T).astype(BF16)
        m["scal"] = np.array(
            [[np.float32(inputs["astrocyte_state"][b]),
              np.float32(np.asarray(inputs["astro_scale"]).reshape(-1)[0])]],
            np.float32)
        in_maps.append(m)
    return in_maps


def kernel(q_in, k_in, v_in, astrocyte_state, Wq, bq, Wk, bk, Wv, bv, Wo, bo,
           role_matrix, Wr1, br1, Wr2, astro_scale, **_ignored):
    nc = _get_nc()
    inputs = dict(q_in=q_in, k_in=k_in, v_in=v_in, astrocyte_state=astrocyte_state,
                  Wq=Wq, bq=bq, Wk=Wk, bk=bk, Wv=Wv, bv=bv, Wo=Wo, bo=bo,
                  role_matrix=role_matrix, Wr1=Wr1, br1=br1, Wr2=Wr2,
                  astro_scale=astro_scale)
    in_maps = build_in_maps(inputs)
    res = run_bass_kernel_spmd(nc, in_maps, core_ids=list(range(8)))

    output = np.empty((B, TQ, D), np.float32)
    new_state = np.empty((B,), np.float32)
    for core in range(8):
        b, h = core // 2, core % 2
        output[b, h * TQH:(h + 1) * TQH, :] = res.results[core]["outT"].T
        if h == 0:
            new_state[b] = res.results[core]["ns_out"][0, 0]
    return output, new_state
